# revision 20
# baseline (speedup 1.0000x reference)
"""Multi-head attention (B=2, S=2048, D=1024, H=16, K=64) on 8 TRN2 cores.

Sharding: core c -> batch b=c//4, head-group g=c%4 (4 heads, 256-wide slice
of Wq/Wk/Wv columns and Wo rows).  Each core computes a partial (2048, 1024)
output in bf16; host sums groups of 4 cores in f32 and adds bo.

Per-core layout (all transposed so no on-chip transposes are needed):
  - host supplies xT = x[b].T  (D, S), bf16
  - Q^T, K^T computed as [gw_col, S] via lhsT=W chunk, rhs=xT chunk
  - scores^T[j, i] via lhsT=K^T chunk, rhs=Q^T into double-buffered 2-bank
    PSUM tiles; one ScalarE Exp covers 1024 elements
  - softmax denominator via a ones column appended to V (V_aug); probs are
    exp(scores/8) with no max subtraction (scores ~N(0,1), no overflow)
  - O^T = V_aug^T @ probs^T; division by the denominator happens on VectorE
    with head-pair batching (one reciprocal per pair, broadcast matmuls to
    the two column groups of one PSUM bank)

Scheduling: the attention inner loop alone cannot keep TensorE busy (the
Exp on ScalarE is the per-iteration rate limiter), and PE micro-idles make
the HAM clock-gate re-throttle the array to 1.2 GHz.  So the Wo matmuls of
the previous i-group and the Q^T projection of the next i-group are pumped
into the PE queue as filler work between score groups, keeping the PE
saturated and warm.  AV matmuls are emitted one score-group behind so the
PE never head-of-line blocks on the Exp.

All matmul operands are bf16 (PSUM accumulation stays fp32).
"""

import os
import sys
from contextlib import ExitStack

import numpy as np

if "/opt/trn_rl_repo" not in sys.path:
    sys.path.insert(0, "/opt/trn_rl_repo")

import concourse.bass as bass
import concourse.mybir as mybir
import concourse.tile as tile
from concourse import bacc
from concourse.bass import ds, ts
from concourse.bass_utils import run_bass_kernel_spmd

B, S, D = 2, 2048, 1024
H, KS = 16, 64
NCORES = 8
HPC = H // 4          # 4 heads per core
GW = HPC * KS         # 256-wide head-group slice
P = 128
ND = D // P           # 8 contraction chunks over d_model
NM = GW // P          # 2 col chunks of the group slice
NI = 4                # i-groups
IT = S // NI          # 512 rows per i-group
NJ = S // P           # 16 j-chunks
NJJ = 2               # j-chunks per Exp batch ([128,1024] ACT, 2 PSUM banks)
NG = NJ // NJJ        # score groups per head
NO = D // 512         # 2 out-col groups for Wo

F32 = mybir.dt.float32
BF16 = mybir.dt.bfloat16
MMDT = BF16
EXP = mybir.ActivationFunctionType.Exp


def _mha_core(tc, out, xT, wq, wk, wv, wo, bq, bk, bv):
    nc = tc.nc
    with ExitStack() as ctx:
        cp = ctx.enter_context(tc.tile_pool(name="const", bufs=1))
        probs_pool = ctx.enter_context(tc.tile_pool(name="probs", bufs=4))
        out_pool = ctx.enter_context(tc.tile_pool(name="outsb", bufs=3))
        den_pool = ctx.enter_context(tc.tile_pool(name="den", bufs=2))

        # ---- ACT table preload: tiny exp before anything else on ScalarE ----
        warm = cp.tile([1, 16], F32)
        nc.vector.memset(warm[:], 0.0)
        nc.scalar.activation(warm[:], warm[:], EXP)

        # HAM warmup operands: junk matmuls keep the PE busy while DMAs
        # land (and during the tail den-chain) so real matmuls run at
        # 2.4 GHz instead of the cold 1.2 GHz
        wu_l = cp.tile([P, P], MMDT)
        wu_r = cp.tile([P, IT], MMDT)
        nc.vector.memset(wu_l[:], 0.0)
        nc.vector.memset(wu_r[:], 0.0)

        # ---- inputs to SBUF; sync ring carries what's needed first ----
        wk_sb = cp.tile([P, ND, GW], MMDT)
        nc.sync.dma_start(wk_sb[:], wk.rearrange("(nd p) n -> p nd n", p=P))
        xT_sb = []
        for c4 in range(4):
            xc = cp.tile([P, ND, IT], MMDT, name=f"xc{c4}")
            nc.sync.dma_start(
                xc[:], xT[:, ts(c4, IT)].rearrange("(nd p) s -> p nd s", p=P))
            xT_sb.append(xc)
        wq_sb = cp.tile([P, ND, GW], MMDT)
        nc.sync.dma_start(wq_sb[:], wq.rearrange("(nd p) n -> p nd n", p=P))
        wv_sb = cp.tile([P, ND, GW], MMDT)
        wo_sb = cp.tile([P, NM, D], MMDT)
        nc.scalar.dma_start(wv_sb[:], wv.rearrange("(nd p) n -> p nd n", p=P))
        nc.scalar.dma_start(wo_sb[:], wo.rearrange("(nm p) n -> p nm n", p=P))
        bq_sb = cp.tile([P, NM], F32)
        bk_sb = cp.tile([P, NM], F32)
        nc.scalar.dma_start(bq_sb[:], bq.rearrange("(m p) -> p m", p=P))
        nc.scalar.dma_start(bk_sb[:], bk.rearrange("(m p) -> p m", p=P))
        bv_bc = cp.tile([P, GW], F32)
        nc.scalar.dma_start(bv_bc[:], bv.partition_broadcast(P))

        QT = cp.tile([P, NM, S], MMDT)
        KT = cp.tile([P, NM, S], MMDT)
        OT = cp.tile([P, NM, S], MMDT)
        # V_aug[:, h, jt, 0:64] = V rows, [:, h, jt, 64] = 1.0 (denominator col)
        V_aug = cp.tile([P, HPC, NJ, KS + 1], MMDT)
        nc.vector.memset(
            V_aug[:, :, :, ds(KS, 1)].rearrange("p h j o -> p (h j o)"), 1.0)

        # ones row for the denominator broadcast matmul (1/den -> 64 rows)
        ones64 = cp.tile([1, KS], MMDT)
        nc.vector.memset(ones64[:], 1.0)

        # ---- projections: K^T (ig-major, chasing the xT DMAs), V, Q^T(0) ----
        with tc.tile_pool(name="ps_acc", bufs=4, space="PSUM") as ps_acc:
            wu_ps = ps_acc.tile([P, IT], F32, tag="acc")
            for _ in range(26):
                nc.tensor.matmul(wu_ps[:], wu_l[:], wu_r[:],
                                 start=True, stop=True)
            wu_sb = cp.tile([1, 1], F32)
            nc.vector.tensor_copy(wu_sb[:], wu_ps[ds(0, 1), ds(0, 1)])
            for ig in range(NI):
                for m in range(NM):
                    kt_ps = ps_acc.tile([P, IT], F32, tag="acc")
                    for dc in range(ND):
                        nc.tensor.matmul(
                            kt_ps[:],
                            wk_sb[:, dc, ts(m, P)],
                            xT_sb[ig][:, dc, :],
                            start=(dc == 0), stop=(dc == ND - 1),
                        )
                    nc.vector.tensor_scalar_add(
                        KT[:, m, ts(ig, IT)], kt_ps[:], bk_sb[:, ds(m, 1)])

            for jt in range(NJ):
                v_ps = ps_acc.tile([P, IT], F32, tag="acc")
                for dc in range(ND):
                    nc.tensor.matmul(
                        v_ps[:, 0:GW],
                        xT_sb[jt // 4][:, dc, ts(jt % 4, P)],
                        wv_sb[:, dc, :],
                        start=(dc == 0), stop=(dc == ND - 1),
                    )
                nc.vector.tensor_add(
                    V_aug[:, :, jt, 0:KS],
                    v_ps[:, 0:GW].rearrange("p (h k) -> p h k", h=HPC),
                    bv_bc[:].rearrange("p (h k) -> p h k", h=HPC),
                )

            for m in range(NM):
                qt_ps = ps_acc.tile([P, IT], F32, tag="acc")
                for dc in range(ND):
                    nc.tensor.matmul(
                        qt_ps[:],
                        wq_sb[:, dc, ts(m, P)],
                        xT_sb[0][:, dc, :],
                        start=(dc == 0), stop=(dc == ND - 1),
                    )
                nc.vector.tensor_scalar_add(
                    QT[:, m, ts(0, IT)], qt_ps[:], bq_sb[:, ds(m, 1)])

        # ---- attention + interleaved Wo / Q^T-projection filler ----
        # PSUM budget: s2 = 2x2, o_ps = 2, misc(w/qt/bc) = 2 shared slots
        with tc.tile_pool(name="ps_s", bufs=2, space="PSUM") as ps_s, \
             tc.tile_pool(name="ps_o", bufs=2, space="PSUM") as ps_o, \
             tc.tile_pool(name="ps_m", bufs=2, space="PSUM") as ps_m:
            ps_c = ps_m

            stage, stage_left = {}, {}

            def wo_unit(it, ncol):
                def emit():
                    g = it // NI
                    if g not in stage:
                        stage[g] = out_pool.tile([P, NI, D], MMDT,
                                                 name="ostage", tag="ost",
                                                 bufs=2)
                        stage_left[g] = NI * NO
                    w_ps = ps_m.tile([P, 512], F32, tag="m")
                    for hc in range(NM):
                        nc.tensor.matmul(
                            w_ps[:],
                            OT[:, hc, ts(it, P)],
                            wo_sb[:, hc, ts(ncol, 512)],
                            start=(hc == 0), stop=(hc == NM - 1),
                        )
                    st = stage[g]
                    nc.vector.tensor_copy(st[:, it % NI, ts(ncol, 512)],
                                          w_ps[:])
                    stage_left[g] -= 1
                    if stage_left[g] == 0:
                        eng = nc.sync if g % 2 == 0 else nc.scalar
                        eng.dma_start(
                            out[ts(g, 4 * P), :].rearrange(
                                "(itl p) d -> p itl d", p=P),
                            st[:])
                        del stage[g]
                return emit

            def qt_units(g, m):
                """Q^T projection of i-group g, col chunk m, as 4 filler
                units of 2 accumulating matmuls each."""
                state = {}

                def unit(k):
                    def emit():
                        if k == 0:
                            state["ps"] = ps_m.tile([P, IT], F32,
                                                    name="qt_fill_ps", tag="m")
                        qt_ps = state["ps"]
                        for dc in (2 * k, 2 * k + 1):
                            nc.tensor.matmul(
                                qt_ps[:],
                                wq_sb[:, dc, ts(m, P)],
                                xT_sb[g][:, dc, :],
                                start=(dc == 0), stop=(dc == ND - 1),
                            )
                        if k == 3:
                            nc.vector.tensor_scalar_add(
                                QT[:, m, ts(g, IT)], qt_ps[:],
                                bq_sb[:, ds(m, 1)])
                    return emit
                return [unit(k) for k in range(4)]

            def den_chain(ig, m, oe_sb, oo_sb, den2):
                """Normalize heads 2m, 2m+1 of i-group ig from their SBUF
                evacuations (o rows in oe_sb/oo_sb, denominators in den2)."""
                recip2 = den_pool.tile([1, 2, IT], F32)
                nc.vector.reciprocal_approx_fast(
                    recip2[:].rearrange("p a b -> p (a b)"),
                    den2[:].rearrange("p a b -> p (a b)"))
                recip2b = den_pool.tile([1, 2, IT], MMDT)
                nc.vector.tensor_copy(
                    recip2b[:].rearrange("p a b -> p (a b)"),
                    recip2[:].rearrange("p a b -> p (a b)"))
                bc_ps = ps_c.tile([P, IT], F32, tag="m")
                nc.tensor.matmul(bc_ps[ds(0, KS), :], ones64[:],
                                 recip2b[:, 0, :], start=True, stop=True)
                nc.tensor.matmul(bc_ps[ds(KS, KS), :], ones64[:],
                                 recip2b[:, 1, :], start=True, stop=True,
                                 tile_position=(0, KS))
                bc_sb = den_pool.tile([KS, 2, IT], F32)
                nc.vector.tensor_copy(bc_sb[:, 0, :], bc_ps[ds(0, KS), :])
                nc.vector.tensor_copy(bc_sb[:, 1, :], bc_ps[ds(KS, KS), :])
                nc.vector.tensor_mul(
                    OT[ds(0, KS), m, ts(ig, IT)], oe_sb[:], bc_sb[:, 0, :])
                nc.vector.tensor_mul(
                    OT[ds(KS, KS), m, ts(ig, IT)], oo_sb[:], bc_sb[:, 1, :])

            # ---- flat software pipeline over (ig, pair, jc) ----
            # The two heads of a pair occupy row groups 0-63 / 64-127, so
            # their score matmuls run CONCURRENTLY in the PE array (row
            # tiling) and one Exp covers both heads' scores.
            o_tiles = {}
            fill01, fill23 = [], []

            def build_fillers(ig):
                f01, f23 = [], []
                if ig == 0:
                    f01 = qt_units(1, 0) + qt_units(1, 1)
                else:
                    f01 = [wo_unit(4 * (ig - 1) + itl, ncol)
                           for itl in range(NI) for ncol in range(NO)]
                    if ig < NI - 1:
                        f23 = qt_units(ig + 1, 0) + qt_units(ig + 1, 1)
                return f01, f23

            def emit_av(pig, pm, pjc, ppt):
                for par in range(2):
                    key = (pig, 2 * pm + par)
                    if key not in o_tiles:
                        o_tiles[key] = ps_o.tile([KS + 1, IT], F32,
                                                 name="o_ps", tag="o")
                    nc.tensor.matmul(
                        o_tiles[key][:], V_aug[:, 2 * pm + par, pjc, :],
                        ppt[:, par, :],
                        start=(pjc == 0), stop=(pjc == NJ - 1),
                    )
                if pjc == NJ - 1:
                    # evacuate both heads to SBUF (fast PSUM release);
                    # the pair normalization chain is deferred so its
                    # broadcast matmul never blocks the PE on the
                    # reciprocal latency
                    den2 = den_pool.tile([1, 2, IT], F32)
                    evs = []
                    for par in range(2):
                        o_full = o_tiles.pop((pig, 2 * pm + par))
                        o_sb = den_pool.tile([KS, IT], F32, name="o_evac",
                                             tag=f"oev{par}", bufs=2)
                        nc.vector.tensor_copy(o_sb[:], o_full[ds(0, KS), :])
                        nc.vector.tensor_copy(den2[:, par, :],
                                              o_full[ds(KS, 1), :])
                        evs.append(o_sb)
                    chain_q.append([2, (pig, pm, evs[0], evs[1], den2)])

            pending = []  # (ig, m, jc, pt), AV emitted at depth 2
            chain_q = []  # deferred pair normalization chains
            for ig in range(NI):
                for f in fill01 + fill23:
                    f()
                fill01, fill23 = build_fillers(ig)
                for pr in range(NM):
                    fillers = fill01 if pr == 0 else fill23
                    for jc in range(NJ):
                        sP = ps_s.tile([P, NJJ, IT], F32, tag="s")
                        nc.tensor.matmul(
                            sP[:, 0, :],
                            KT[ds(0, KS), pr, ts(jc, P)],
                            QT[ds(0, KS), pr, ts(ig, IT)],
                            start=True, stop=True,
                        )
                        nc.tensor.matmul(
                            sP[:, 1, :],
                            KT[ds(KS, KS), pr, ts(jc, P)],
                            QT[ds(KS, KS), pr, ts(ig, IT)],
                            start=True, stop=True,
                        )
                        if len(pending) >= 2:
                            emit_av(*pending.pop(0))
                        if chain_q:
                            chain_q[0][0] -= 1
                            if chain_q[0][0] <= 0:
                                den_chain(*chain_q.pop(0)[1])
                        # Wo fillers read OT, written by the deferred chain
                        # (emitted by jc==3 of the next pair) -> pump late
                        if jc % 2 == 1 and jc >= 3 and fillers:
                            fillers.pop(0)()
                            if jc == NJ - 1 and fillers:
                                fillers.pop(0)()
                        pt = probs_pool.tile([P, NJJ, IT], MMDT)
                        nc.scalar.activation(
                            pt[:].rearrange("p a b -> p (a b)"),
                            sP[:].rearrange("p a b -> p (a b)"),
                            EXP, scale=0.125)
                        pending.append((ig, pr, jc, pt))
            # drain; junk matmuls keep the PE warm through the last chain
            wu_ps2 = ps_s.tile([P, NJJ, IT], F32, name="wu_ps2", tag="s")
            emit_av(*pending.pop(0))
            for _ in range(8):
                nc.tensor.matmul(wu_ps2[:, 0, :], wu_l[:], wu_r[:],
                                 start=True, stop=True)
            emit_av(*pending.pop(0))
            while chain_q:
                den_chain(*chain_q.pop(0)[1])
            for _ in range(12):
                nc.tensor.matmul(wu_ps2[:, 0, :], wu_l[:], wu_r[:],
                                 start=True, stop=True)
            nc.vector.tensor_copy(wu_sb[:], wu_ps2[ds(0, 1), 0, ds(0, 1)])
            for f in fill01 + fill23:
                f()

            # tail: Wo of the last i-group
            for itl in range(NI):
                it = 4 * (NI - 1) + itl
                for ncol in range(NO):
                    wo_unit(it, ncol)()


def _build_program():
    nc = bacc.Bacc("TRN2", target_bir_lowering=False, debug=False,
                   num_devices=NCORES)
    xT = nc.dram_tensor("xT", (D, S), MMDT, kind="ExternalInput").ap()
    wq = nc.dram_tensor("wq", (D, GW), MMDT, kind="ExternalInput").ap()
    wk = nc.dram_tensor("wk", (D, GW), MMDT, kind="ExternalInput").ap()
    wv = nc.dram_tensor("wv", (D, GW), MMDT, kind="ExternalInput").ap()
    wo = nc.dram_tensor("wo", (GW, D), MMDT, kind="ExternalInput").ap()
    bq = nc.dram_tensor("bq", (GW,), F32, kind="ExternalInput").ap()
    bk = nc.dram_tensor("bk", (GW,), F32, kind="ExternalInput").ap()
    bv = nc.dram_tensor("bv", (GW,), F32, kind="ExternalInput").ap()
    out = nc.dram_tensor("out", (S, D), MMDT, kind="ExternalOutput").ap()
    with tile.TileContext(nc) as tc:
        _mha_core(tc, out, xT, wq, wk, wv, wo, bq, bk, bv)
    nc.compile()
    return nc


_program = None


def _get_program():
    global _program
    if _program is None:
        _program = _build_program()
    return _program


def make_in_maps(x, Wq, bq, Wk, bk, Wv, bv, Wo, bo):
    in_maps = []
    f = np.float32
    bf = mybir.dt.np(MMDT)
    for c in range(NCORES):
        b, g = divmod(c, 4)
        sl = slice(g * GW, (g + 1) * GW)
        in_maps.append({
            "xT": np.ascontiguousarray(x[b].T).astype(bf),
            "wq": np.ascontiguousarray(Wq[:, sl]).astype(bf),
            "wk": np.ascontiguousarray(Wk[:, sl]).astype(bf),
            "wv": np.ascontiguousarray(Wv[:, sl]).astype(bf),
            "wo": np.ascontiguousarray(Wo[sl, :]).astype(bf),
            "bq": np.ascontiguousarray(bq[sl], dtype=f),
            "bk": np.ascontiguousarray(bk[sl], dtype=f),
            "bv": np.ascontiguousarray(bv[sl], dtype=f),
        })
    return in_maps


def run(inputs, trace=False, tmpdir=None, **kw):
    nc = _get_program()
    in_maps = make_in_maps(**inputs)
    res = run_bass_kernel_spmd(nc, in_maps, core_ids=list(range(NCORES)),
                               trace=trace, tmpdir=tmpdir, **kw)
    bo = inputs["bo"].astype(np.float32)
    parts = [np.asarray(res.results[c]["out"], dtype=np.float32)
             for c in range(NCORES)]
    y = np.stack(
        [parts[4 * b] + parts[4 * b + 1] + parts[4 * b + 2] + parts[4 * b + 3] + bo
         for b in range(B)], axis=0)
    return y.astype(np.float32), res


def kernel(**inputs):
    y, _ = run(inputs, trace=False)
    return y


# revision 21
# speedup vs baseline: 1.0154x; 1.0154x over previous
"""Multi-head attention (B=2, S=2048, D=1024, H=16, K=64) on 8 TRN2 cores.

Sharding: core c -> batch b=c//4, head-group g=c%4 (4 heads, 256-wide slice
of Wq/Wk/Wv columns and Wo rows).  Each core computes a partial (2048, 1024)
output in bf16; host sums groups of 4 cores in f32 and adds bo.

Per-core layout (all transposed so no on-chip transposes are needed):
  - host supplies xT = x[b].T  (D, S), bf16
  - Q^T, K^T computed as [gw_col, S] via lhsT=W chunk, rhs=xT chunk
  - scores^T[j, i] via lhsT=K^T chunk, rhs=Q^T into double-buffered 2-bank
    PSUM tiles; one ScalarE Exp covers 1024 elements
  - softmax denominator via a ones column appended to V (V_aug); probs are
    exp(scores/8) with no max subtraction (scores ~N(0,1), no overflow)
  - O^T = V_aug^T @ probs^T; division by the denominator happens on VectorE
    with head-pair batching (one reciprocal per pair, broadcast matmuls to
    the two column groups of one PSUM bank)

Scheduling: the attention inner loop alone cannot keep TensorE busy (the
Exp on ScalarE is the per-iteration rate limiter), and PE micro-idles make
the HAM clock-gate re-throttle the array to 1.2 GHz.  So the Wo matmuls of
the previous i-group and the Q^T projection of the next i-group are pumped
into the PE queue as filler work between score groups, keeping the PE
saturated and warm.  AV matmuls are emitted one score-group behind so the
PE never head-of-line blocks on the Exp.

All matmul operands are bf16 (PSUM accumulation stays fp32).
"""

import os
import sys
from contextlib import ExitStack

import numpy as np

if "/opt/trn_rl_repo" not in sys.path:
    sys.path.insert(0, "/opt/trn_rl_repo")

import concourse.bass as bass
import concourse.mybir as mybir
import concourse.tile as tile
from concourse import bacc
from concourse.bass import ds, ts
from concourse.bass_utils import run_bass_kernel_spmd

B, S, D = 2, 2048, 1024
H, KS = 16, 64
NCORES = 8
HPC = H // 4          # 4 heads per core
GW = HPC * KS         # 256-wide head-group slice
P = 128
ND = D // P           # 8 contraction chunks over d_model
NM = GW // P          # 2 col chunks of the group slice
NI = 4                # i-groups
IT = S // NI          # 512 rows per i-group
NJ = S // P           # 16 j-chunks
NJJ = 2               # j-chunks per Exp batch ([128,1024] ACT, 2 PSUM banks)
NG = NJ // NJJ        # score groups per head
NO = D // 512         # 2 out-col groups for Wo

F32 = mybir.dt.float32
BF16 = mybir.dt.bfloat16
MMDT = BF16
EXP = mybir.ActivationFunctionType.Exp


def _mha_core(tc, out, xT, wq, wk, wv, wo, bq, bk, bv):
    nc = tc.nc
    with ExitStack() as ctx:
        cp = ctx.enter_context(tc.tile_pool(name="const", bufs=1))
        probs_pool = ctx.enter_context(tc.tile_pool(name="probs", bufs=5))
        out_pool = ctx.enter_context(tc.tile_pool(name="outsb", bufs=3))
        den_pool = ctx.enter_context(tc.tile_pool(name="den", bufs=2))

        # ---- ACT table preload: tiny exp before anything else on ScalarE ----
        warm = cp.tile([1, 16], F32)
        nc.vector.memset(warm[:], 0.0)
        nc.scalar.activation(warm[:], warm[:], EXP)

        # HAM warmup operands: junk matmuls keep the PE busy while DMAs
        # land (and during the tail den-chain) so real matmuls run at
        # 2.4 GHz instead of the cold 1.2 GHz
        wu_l = cp.tile([P, P], MMDT)
        wu_r = cp.tile([P, IT], MMDT)
        nc.vector.memset(wu_l[:], 0.0)
        nc.vector.memset(wu_r[:], 0.0)

        # ---- inputs to SBUF; sync ring carries what's needed first ----
        wk_sb = cp.tile([P, ND, GW], MMDT)
        nc.sync.dma_start(wk_sb[:], wk.rearrange("(nd p) n -> p nd n", p=P))
        xT_sb = []
        for c4 in range(4):
            xc = cp.tile([P, ND, IT], MMDT, name=f"xc{c4}")
            nc.sync.dma_start(
                xc[:], xT[:, ts(c4, IT)].rearrange("(nd p) s -> p nd s", p=P))
            xT_sb.append(xc)
        wq_sb = cp.tile([P, ND, GW], MMDT)
        nc.sync.dma_start(wq_sb[:], wq.rearrange("(nd p) n -> p nd n", p=P))
        wv_sb = cp.tile([P, ND, GW], MMDT)
        wo_sb = cp.tile([P, NM, D], MMDT)
        nc.scalar.dma_start(wv_sb[:], wv.rearrange("(nd p) n -> p nd n", p=P))
        nc.scalar.dma_start(wo_sb[:], wo.rearrange("(nm p) n -> p nm n", p=P))
        bq_sb = cp.tile([P, NM], F32)
        bk_sb = cp.tile([P, NM], F32)
        nc.scalar.dma_start(bq_sb[:], bq.rearrange("(m p) -> p m", p=P))
        nc.scalar.dma_start(bk_sb[:], bk.rearrange("(m p) -> p m", p=P))
        bv_bc = cp.tile([P, GW], F32)
        nc.scalar.dma_start(bv_bc[:], bv.partition_broadcast(P))

        QT = cp.tile([P, NM, S], MMDT)
        KT = cp.tile([P, NM, S], MMDT)
        OT = cp.tile([P, NM, S], MMDT)
        # V_aug[:, h, jt, 0:64] = V rows, [:, h, jt, 64] = 1.0 (denominator col)
        V_aug = cp.tile([P, HPC, NJ, KS + 1], MMDT)
        nc.vector.memset(
            V_aug[:, :, :, ds(KS, 1)].rearrange("p h j o -> p (h j o)"), 1.0)

        # ones row for the denominator broadcast matmul (1/den -> 64 rows)
        ones64 = cp.tile([1, KS], MMDT)
        nc.vector.memset(ones64[:], 1.0)

        # ---- projections: K^T (ig-major, chasing the xT DMAs), V, Q^T(0) ----
        with tc.tile_pool(name="ps_acc", bufs=4, space="PSUM") as ps_acc:
            wu_ps = ps_acc.tile([P, IT], F32, tag="acc")
            for _ in range(16):
                nc.tensor.matmul(wu_ps[:], wu_l[:], wu_r[:],
                                 start=True, stop=True)
            wu_sb = cp.tile([1, 1], F32)
            nc.vector.tensor_copy(wu_sb[:], wu_ps[ds(0, 1), ds(0, 1)])
            for ig in range(NI):
                for m in range(NM):
                    kt_ps = ps_acc.tile([P, IT], F32, tag="acc")
                    for dc in range(ND):
                        nc.tensor.matmul(
                            kt_ps[:],
                            wk_sb[:, dc, ts(m, P)],
                            xT_sb[ig][:, dc, :],
                            start=(dc == 0), stop=(dc == ND - 1),
                        )
                    nc.vector.tensor_scalar_add(
                        KT[:, m, ts(ig, IT)], kt_ps[:], bk_sb[:, ds(m, 1)])

            for jt in range(NJ):
                v_ps = ps_acc.tile([P, IT], F32, tag="acc")
                for dc in range(ND):
                    nc.tensor.matmul(
                        v_ps[:, 0:GW],
                        xT_sb[jt // 4][:, dc, ts(jt % 4, P)],
                        wv_sb[:, dc, :],
                        start=(dc == 0), stop=(dc == ND - 1),
                    )
                nc.vector.tensor_add(
                    V_aug[:, :, jt, 0:KS],
                    v_ps[:, 0:GW].rearrange("p (h k) -> p h k", h=HPC),
                    bv_bc[:].rearrange("p (h k) -> p h k", h=HPC),
                )

            for m in range(NM):
                qt_ps = ps_acc.tile([P, IT], F32, tag="acc")
                for dc in range(ND):
                    nc.tensor.matmul(
                        qt_ps[:],
                        wq_sb[:, dc, ts(m, P)],
                        xT_sb[0][:, dc, :],
                        start=(dc == 0), stop=(dc == ND - 1),
                    )
                nc.vector.tensor_scalar_add(
                    QT[:, m, ts(0, IT)], qt_ps[:], bq_sb[:, ds(m, 1)])

        # ---- attention + interleaved Wo / Q^T-projection filler ----
        # PSUM budget: s2 = 2x2, o_ps = 2, misc(w/qt/bc) = 2 shared slots
        with tc.tile_pool(name="ps_s", bufs=2, space="PSUM") as ps_s, \
             tc.tile_pool(name="ps_o", bufs=2, space="PSUM") as ps_o, \
             tc.tile_pool(name="ps_m", bufs=2, space="PSUM") as ps_m:
            ps_c = ps_m

            stage, stage_left = {}, {}

            def wo_unit(it, ncol):
                def emit():
                    g = it // NI
                    if g not in stage and g != NI - 1:
                        stage[g] = out_pool.tile([P, NI, D], MMDT,
                                                 name="ostage", tag="ost",
                                                 bufs=2)
                        stage_left[g] = NI * NO
                    w_ps = ps_m.tile([P, 512], F32, tag="m")
                    for hc in range(NM):
                        nc.tensor.matmul(
                            w_ps[:],
                            OT[:, hc, ts(it, P)],
                            wo_sb[:, hc, ts(ncol, 512)],
                            start=(hc == 0), stop=(hc == NM - 1),
                        )
                    if g == NI - 1:
                        o_sb = out_pool.tile([P, 512], MMDT)
                        nc.vector.tensor_copy(o_sb[:], w_ps[:])
                        eng = nc.sync if (it + ncol) % 2 == 0 else nc.scalar
                        eng.dma_start(out[ts(it, P), ts(ncol, 512)], o_sb[:])
                        return
                    st = stage[g]
                    nc.vector.tensor_copy(st[:, it % NI, ts(ncol, 512)],
                                          w_ps[:])
                    stage_left[g] -= 1
                    if stage_left[g] == 0:
                        eng = nc.sync if g % 2 == 0 else nc.scalar
                        eng.dma_start(
                            out[ts(g, 4 * P), :].rearrange(
                                "(itl p) d -> p itl d", p=P),
                            st[:])
                        del stage[g]
                return emit

            def qt_units(g, m):
                """Q^T projection of i-group g, col chunk m, as 4 filler
                units of 2 accumulating matmuls each."""
                state = {}

                def unit(k):
                    def emit():
                        if k == 0:
                            state["ps"] = ps_m.tile([P, IT], F32,
                                                    name="qt_fill_ps", tag="m")
                        qt_ps = state["ps"]
                        for dc in (2 * k, 2 * k + 1):
                            nc.tensor.matmul(
                                qt_ps[:],
                                wq_sb[:, dc, ts(m, P)],
                                xT_sb[g][:, dc, :],
                                start=(dc == 0), stop=(dc == ND - 1),
                            )
                        if k == 3:
                            nc.vector.tensor_scalar_add(
                                QT[:, m, ts(g, IT)], qt_ps[:],
                                bq_sb[:, ds(m, 1)])
                    return emit
                return [unit(k) for k in range(4)]

            def den_chain(ig, m, oe_sb, oo_sb, den2):
                """Normalize heads 2m, 2m+1 of i-group ig from their SBUF
                evacuations (o rows in oe_sb/oo_sb, denominators in den2)."""
                recip2 = den_pool.tile([1, 2, IT], F32)
                nc.vector.reciprocal_approx_fast(
                    recip2[:].rearrange("p a b -> p (a b)"),
                    den2[:].rearrange("p a b -> p (a b)"))
                recip2b = den_pool.tile([1, 2, IT], MMDT)
                nc.vector.tensor_copy(
                    recip2b[:].rearrange("p a b -> p (a b)"),
                    recip2[:].rearrange("p a b -> p (a b)"))
                bc_ps = ps_c.tile([P, IT], F32, tag="m")
                nc.tensor.matmul(bc_ps[ds(0, KS), :], ones64[:],
                                 recip2b[:, 0, :], start=True, stop=True)
                nc.tensor.matmul(bc_ps[ds(KS, KS), :], ones64[:],
                                 recip2b[:, 1, :], start=True, stop=True,
                                 tile_position=(0, KS))
                bc_sb = den_pool.tile([KS, 2, IT], F32)
                nc.vector.tensor_copy(bc_sb[:, 0, :], bc_ps[ds(0, KS), :])
                nc.vector.tensor_copy(bc_sb[:, 1, :], bc_ps[ds(KS, KS), :])
                nc.vector.tensor_mul(
                    OT[ds(0, KS), m, ts(ig, IT)], oe_sb[:], bc_sb[:, 0, :])
                nc.vector.tensor_mul(
                    OT[ds(KS, KS), m, ts(ig, IT)], oo_sb[:], bc_sb[:, 1, :])

            # ---- flat software pipeline over (ig, pair, jc) ----
            # The two heads of a pair occupy row groups 0-63 / 64-127, so
            # their score matmuls run CONCURRENTLY in the PE array (row
            # tiling) and one Exp covers both heads' scores.
            o_tiles = {}
            fill01, fill23 = [], []

            def build_fillers(ig):
                f01, f23 = [], []
                if ig == 0:
                    f01 = qt_units(1, 0) + qt_units(1, 1)
                else:
                    f01 = [wo_unit(4 * (ig - 1) + itl, ncol)
                           for itl in range(NI) for ncol in range(NO)]
                    if ig < NI - 1:
                        f23 = qt_units(ig + 1, 0) + qt_units(ig + 1, 1)
                return f01, f23

            def emit_av(pig, pm, pjc, ppt):
                for par in range(2):
                    key = (pig, 2 * pm + par)
                    if key not in o_tiles:
                        o_tiles[key] = ps_o.tile([KS + 1, IT], F32,
                                                 name="o_ps", tag="o")
                    nc.tensor.matmul(
                        o_tiles[key][:], V_aug[:, 2 * pm + par, pjc, :],
                        ppt[:, par, :],
                        start=(pjc == 0), stop=(pjc == NJ - 1),
                    )
                if pjc == NJ - 1:
                    # evacuate both heads to SBUF (fast PSUM release);
                    # the pair normalization chain is deferred so its
                    # broadcast matmul never blocks the PE on the
                    # reciprocal latency
                    den2 = den_pool.tile([1, 2, IT], F32)
                    evs = []
                    for par in range(2):
                        o_full = o_tiles.pop((pig, 2 * pm + par))
                        o_sb = den_pool.tile([KS, IT], F32, name="o_evac",
                                             tag=f"oev{par}", bufs=2)
                        nc.vector.tensor_copy(o_sb[:], o_full[ds(0, KS), :])
                        nc.vector.tensor_copy(den2[:, par, :],
                                              o_full[ds(KS, 1), :])
                        evs.append(o_sb)
                    chain_q.append([2, (pig, pm, evs[0], evs[1], den2)])

            pending = []  # (ig, m, jc, pt), AV emitted at depth 2
            chain_q = []  # deferred pair normalization chains
            for ig in range(NI):
                for f in fill01 + fill23:
                    f()
                fill01, fill23 = build_fillers(ig)
                for pr in range(NM):
                    fillers = fill01 if pr == 0 else fill23
                    for jc in range(NJ):
                        sP = ps_s.tile([P, NJJ, IT], F32, tag="s")
                        nc.tensor.matmul(
                            sP[:, 0, :],
                            KT[ds(0, KS), pr, ts(jc, P)],
                            QT[ds(0, KS), pr, ts(ig, IT)],
                            start=True, stop=True,
                        )
                        nc.tensor.matmul(
                            sP[:, 1, :],
                            KT[ds(KS, KS), pr, ts(jc, P)],
                            QT[ds(KS, KS), pr, ts(ig, IT)],
                            start=True, stop=True,
                        )
                        if len(pending) >= 3:
                            emit_av(*pending.pop(0))
                        if chain_q:
                            chain_q[0][0] -= 1
                            if chain_q[0][0] <= 0:
                                den_chain(*chain_q.pop(0)[1])
                        # Wo fillers read OT, written by the deferred chain
                        # (emitted by jc==3 of the next pair) -> pump late
                        if jc % 2 == 1 and jc >= 3 and fillers:
                            fillers.pop(0)()
                            if jc == NJ - 1 and fillers:
                                fillers.pop(0)()
                        pt = probs_pool.tile([P, NJJ, IT], MMDT)
                        nc.scalar.activation(
                            pt[:].rearrange("p a b -> p (a b)"),
                            sP[:].rearrange("p a b -> p (a b)"),
                            EXP, scale=0.125)
                        pending.append((ig, pr, jc, pt))
            # drain; junk matmuls keep the PE warm through the last chain
            wu_ps2 = ps_s.tile([P, NJJ, IT], F32, name="wu_ps2", tag="s")
            emit_av(*pending.pop(0))
            for _ in range(4):
                nc.tensor.matmul(wu_ps2[:, 0, :], wu_l[:], wu_r[:],
                                 start=True, stop=True)
            emit_av(*pending.pop(0))
            for _ in range(4):
                nc.tensor.matmul(wu_ps2[:, 0, :], wu_l[:], wu_r[:],
                                 start=True, stop=True)
            emit_av(*pending.pop(0))
            while chain_q:
                den_chain(*chain_q.pop(0)[1])
            for _ in range(12):
                nc.tensor.matmul(wu_ps2[:, 0, :], wu_l[:], wu_r[:],
                                 start=True, stop=True)
            nc.vector.tensor_copy(wu_sb[:], wu_ps2[ds(0, 1), 0, ds(0, 1)])
            for f in fill01 + fill23:
                f()

            # tail: Wo of the last i-group
            for itl in range(NI):
                it = 4 * (NI - 1) + itl
                for ncol in range(NO):
                    wo_unit(it, ncol)()


def _build_program():
    nc = bacc.Bacc("TRN2", target_bir_lowering=False, debug=False,
                   num_devices=NCORES)
    xT = nc.dram_tensor("xT", (D, S), MMDT, kind="ExternalInput").ap()
    wq = nc.dram_tensor("wq", (D, GW), MMDT, kind="ExternalInput").ap()
    wk = nc.dram_tensor("wk", (D, GW), MMDT, kind="ExternalInput").ap()
    wv = nc.dram_tensor("wv", (D, GW), MMDT, kind="ExternalInput").ap()
    wo = nc.dram_tensor("wo", (GW, D), MMDT, kind="ExternalInput").ap()
    bq = nc.dram_tensor("bq", (GW,), F32, kind="ExternalInput").ap()
    bk = nc.dram_tensor("bk", (GW,), F32, kind="ExternalInput").ap()
    bv = nc.dram_tensor("bv", (GW,), F32, kind="ExternalInput").ap()
    out = nc.dram_tensor("out", (S, D), MMDT, kind="ExternalOutput").ap()
    with tile.TileContext(nc) as tc:
        _mha_core(tc, out, xT, wq, wk, wv, wo, bq, bk, bv)
    nc.compile()
    return nc


_program = None


def _get_program():
    global _program
    if _program is None:
        _program = _build_program()
    return _program


def make_in_maps(x, Wq, bq, Wk, bk, Wv, bv, Wo, bo):
    in_maps = []
    f = np.float32
    bf = mybir.dt.np(MMDT)
    for c in range(NCORES):
        b, g = divmod(c, 4)
        sl = slice(g * GW, (g + 1) * GW)
        in_maps.append({
            "xT": np.ascontiguousarray(x[b].T).astype(bf),
            "wq": np.ascontiguousarray(Wq[:, sl]).astype(bf),
            "wk": np.ascontiguousarray(Wk[:, sl]).astype(bf),
            "wv": np.ascontiguousarray(Wv[:, sl]).astype(bf),
            "wo": np.ascontiguousarray(Wo[sl, :]).astype(bf),
            "bq": np.ascontiguousarray(bq[sl], dtype=f),
            "bk": np.ascontiguousarray(bk[sl], dtype=f),
            "bv": np.ascontiguousarray(bv[sl], dtype=f),
        })
    return in_maps


def run(inputs, trace=False, tmpdir=None, **kw):
    nc = _get_program()
    in_maps = make_in_maps(**inputs)
    res = run_bass_kernel_spmd(nc, in_maps, core_ids=list(range(NCORES)),
                               trace=trace, tmpdir=tmpdir, **kw)
    bo = inputs["bo"].astype(np.float32)
    parts = [np.asarray(res.results[c]["out"], dtype=np.float32)
             for c in range(NCORES)]
    y = np.stack(
        [parts[4 * b] + parts[4 * b + 1] + parts[4 * b + 2] + parts[4 * b + 3] + bo
         for b in range(B)], axis=0)
    return y.astype(np.float32), res


def kernel(**inputs):
    y, _ = run(inputs, trace=False)
    return y


# revision 22
# speedup vs baseline: 1.0279x; 1.0123x over previous
"""Multi-head attention (B=2, S=2048, D=1024, H=16, K=64) on 8 TRN2 cores.

Sharding: core c -> batch b=c//4, head-group g=c%4 (4 heads, 256-wide slice
of Wq/Wk/Wv columns and Wo rows).  Each core computes a partial (2048, 1024)
output in bf16; host sums groups of 4 cores in f32 and adds bo.

Per-core layout (all transposed so no on-chip transposes are needed):
  - host supplies xT = x[b].T  (D, S), bf16
  - Q^T, K^T computed as [gw_col, S] via lhsT=W chunk, rhs=xT chunk
  - scores^T[j, i] via lhsT=K^T chunk, rhs=Q^T into double-buffered 2-bank
    PSUM tiles; one ScalarE Exp covers 1024 elements
  - softmax denominator via a ones column appended to V (V_aug); probs are
    exp(scores/8) with no max subtraction (scores ~N(0,1), no overflow)
  - O^T = V_aug^T @ probs^T; division by the denominator happens on VectorE
    with head-pair batching (one reciprocal per pair, broadcast matmuls to
    the two column groups of one PSUM bank)

Scheduling: the attention inner loop alone cannot keep TensorE busy (the
Exp on ScalarE is the per-iteration rate limiter), and PE micro-idles make
the HAM clock-gate re-throttle the array to 1.2 GHz.  So the Wo matmuls of
the previous i-group and the Q^T projection of the next i-group are pumped
into the PE queue as filler work between score groups, keeping the PE
saturated and warm.  AV matmuls are emitted one score-group behind so the
PE never head-of-line blocks on the Exp.

All matmul operands are bf16 (PSUM accumulation stays fp32).
"""

import os
import sys
from contextlib import ExitStack

import numpy as np

if "/opt/trn_rl_repo" not in sys.path:
    sys.path.insert(0, "/opt/trn_rl_repo")

import concourse.bass as bass
import concourse.mybir as mybir
import concourse.tile as tile
from concourse import bacc
from concourse.bass import ds, ts
from concourse.bass_utils import run_bass_kernel_spmd

B, S, D = 2, 2048, 1024
H, KS = 16, 64
NCORES = 8
HPC = H // 4          # 4 heads per core
GW = HPC * KS         # 256-wide head-group slice
P = 128
ND = D // P           # 8 contraction chunks over d_model
NM = GW // P          # 2 col chunks of the group slice
NI = 4                # i-groups
IT = S // NI          # 512 rows per i-group
NJ = S // P           # 16 j-chunks
NJJ = 2               # j-chunks per Exp batch ([128,1024] ACT, 2 PSUM banks)
NG = NJ // NJJ        # score groups per head
NO = D // 512         # 2 out-col groups for Wo

F32 = mybir.dt.float32
BF16 = mybir.dt.bfloat16
MMDT = BF16
EXP = mybir.ActivationFunctionType.Exp


def _mha_core(tc, out, xT, wq, wk, wv, wo, bq, bk, bv):
    nc = tc.nc
    with ExitStack() as ctx:
        cp = ctx.enter_context(tc.tile_pool(name="const", bufs=1))
        probs_pool = ctx.enter_context(tc.tile_pool(name="probs", bufs=5))
        out_pool = ctx.enter_context(tc.tile_pool(name="outsb", bufs=3))
        den_pool = ctx.enter_context(tc.tile_pool(name="den", bufs=2))

        # ---- ACT table preload: tiny exp before anything else on ScalarE ----
        warm = cp.tile([1, 16], F32)
        nc.vector.memset(warm[:], 0.0)
        nc.scalar.activation(warm[:], warm[:], EXP)

        # HAM warmup operands: junk matmuls keep the PE busy while DMAs
        # land (and during the tail den-chain) so real matmuls run at
        # 2.4 GHz instead of the cold 1.2 GHz
        wu_l = cp.tile([P, P], MMDT)
        wu_r = cp.tile([P, IT], MMDT)
        nc.vector.memset(wu_l[:], 0.0)
        nc.vector.memset(wu_r[:], 0.0)

        # ---- inputs to SBUF; sync ring carries what's needed first ----
        wk_sb = cp.tile([P, ND, GW], MMDT)
        nc.sync.dma_start(wk_sb[:], wk.rearrange("(nd p) n -> p nd n", p=P))
        xT_sb = []
        for c4 in range(4):
            xc = cp.tile([P, ND, IT], MMDT, name=f"xc{c4}")
            nc.sync.dma_start(
                xc[:], xT[:, ts(c4, IT)].rearrange("(nd p) s -> p nd s", p=P))
            xT_sb.append(xc)
        wq_sb = cp.tile([P, ND, GW], MMDT)
        nc.sync.dma_start(wq_sb[:], wq.rearrange("(nd p) n -> p nd n", p=P))
        wv_sb = cp.tile([P, ND, GW], MMDT)
        wo_sb = cp.tile([P, NM, D], MMDT)
        nc.scalar.dma_start(wv_sb[:], wv.rearrange("(nd p) n -> p nd n", p=P))
        nc.scalar.dma_start(wo_sb[:], wo.rearrange("(nm p) n -> p nm n", p=P))
        bq_sb = cp.tile([P, NM], F32)
        bk_sb = cp.tile([P, NM], F32)
        nc.scalar.dma_start(bq_sb[:], bq.rearrange("(m p) -> p m", p=P))
        nc.scalar.dma_start(bk_sb[:], bk.rearrange("(m p) -> p m", p=P))
        bv_bc = cp.tile([P, GW], F32)
        nc.scalar.dma_start(bv_bc[:], bv.partition_broadcast(P))

        QT = cp.tile([P, NM, S], MMDT)
        KT = cp.tile([P, NM, S], MMDT)
        OT = cp.tile([P, NM, S], MMDT)
        # V_aug[:, h, jt, 0:64] = V rows, [:, h, jt, 64] = 1.0 (denominator col)
        V_aug = cp.tile([P, HPC, NJ, KS + 1], MMDT)
        nc.vector.memset(
            V_aug[:, :, :, ds(KS, 1)].rearrange("p h j o -> p (h j o)"), 1.0)

        # ones row for the denominator broadcast matmul (1/den -> 64 rows)
        ones64 = cp.tile([1, KS], MMDT)
        nc.vector.memset(ones64[:], 1.0)

        # ---- projections: K^T (ig-major, chasing the xT DMAs), V, Q^T(0) ----
        with tc.tile_pool(name="ps_acc", bufs=4, space="PSUM") as ps_acc:
            wu_ps = ps_acc.tile([P, IT], F32, tag="acc")
            for _ in range(16):
                nc.tensor.matmul(wu_ps[:], wu_l[:], wu_r[:],
                                 start=True, stop=True)
            wu_sb = cp.tile([1, 1], F32)
            nc.vector.tensor_copy(wu_sb[:], wu_ps[ds(0, 1), ds(0, 1)])
            for ig in range(NI):
                for m in range(NM):
                    kt_ps = ps_acc.tile([P, IT], F32, tag="acc")
                    for dc in range(ND):
                        nc.tensor.matmul(
                            kt_ps[:],
                            wk_sb[:, dc, ts(m, P)],
                            xT_sb[ig][:, dc, :],
                            start=(dc == 0), stop=(dc == ND - 1),
                        )
                    nc.vector.tensor_scalar_add(
                        KT[:, m, ts(ig, IT)], kt_ps[:], bk_sb[:, ds(m, 1)])

            for jt in range(NJ):
                v_ps = ps_acc.tile([P, IT], F32, tag="acc")
                for dc in range(ND):
                    nc.tensor.matmul(
                        v_ps[:, 0:GW],
                        xT_sb[jt // 4][:, dc, ts(jt % 4, P)],
                        wv_sb[:, dc, :],
                        start=(dc == 0), stop=(dc == ND - 1),
                    )
                nc.vector.tensor_add(
                    V_aug[:, :, jt, 0:KS],
                    v_ps[:, 0:GW].rearrange("p (h k) -> p h k", h=HPC),
                    bv_bc[:].rearrange("p (h k) -> p h k", h=HPC),
                )

            for m in range(NM):
                qt_ps = ps_acc.tile([P, IT], F32, tag="acc")
                for dc in range(ND):
                    nc.tensor.matmul(
                        qt_ps[:],
                        wq_sb[:, dc, ts(m, P)],
                        xT_sb[0][:, dc, :],
                        start=(dc == 0), stop=(dc == ND - 1),
                    )
                nc.vector.tensor_scalar_add(
                    QT[:, m, ts(0, IT)], qt_ps[:], bq_sb[:, ds(m, 1)])

        # ---- attention + interleaved Wo / Q^T-projection filler ----
        # PSUM budget: s2 = 2x2, o_ps = 2, misc(w/qt/bc) = 2 shared slots
        with tc.tile_pool(name="ps_s", bufs=2, space="PSUM") as ps_s, \
             tc.tile_pool(name="ps_o", bufs=2, space="PSUM") as ps_o, \
             tc.tile_pool(name="ps_m", bufs=2, space="PSUM") as ps_m:
            ps_c = ps_m

            stage, stage_left = {}, {}

            def wo_unit(it, ncol):
                def emit():
                    g = it // NI
                    if g not in stage and g != NI - 1:
                        stage[g] = out_pool.tile([P, NI, D], MMDT,
                                                 name="ostage", tag="ost",
                                                 bufs=2)
                        stage_left[g] = NI * NO
                    w_ps = ps_m.tile([P, 512], F32, tag="m")
                    for hc in range(NM):
                        nc.tensor.matmul(
                            w_ps[:],
                            OT[:, hc, ts(it, P)],
                            wo_sb[:, hc, ts(ncol, 512)],
                            start=(hc == 0), stop=(hc == NM - 1),
                        )
                    if g == NI - 1:
                        o_sb = out_pool.tile([P, 512], MMDT)
                        nc.vector.tensor_copy(o_sb[:], w_ps[:])
                        eng = nc.sync if (it + ncol) % 2 == 0 else nc.scalar
                        eng.dma_start(out[ts(it, P), ts(ncol, 512)], o_sb[:])
                        return
                    st = stage[g]
                    nc.vector.tensor_copy(st[:, it % NI, ts(ncol, 512)],
                                          w_ps[:])
                    stage_left[g] -= 1
                    if stage_left[g] == 0:
                        eng = nc.sync if g % 2 == 0 else nc.scalar
                        eng.dma_start(
                            out[ts(g, 4 * P), :].rearrange(
                                "(itl p) d -> p itl d", p=P),
                            st[:])
                        del stage[g]
                return emit

            def qt_units(g, m):
                """Q^T projection of i-group g, col chunk m, as 4 filler
                units of 2 accumulating matmuls each."""
                state = {}

                def unit(k):
                    def emit():
                        if k == 0:
                            state["ps"] = ps_m.tile([P, IT], F32,
                                                    name="qt_fill_ps", tag="m")
                        qt_ps = state["ps"]
                        for dc in (2 * k, 2 * k + 1):
                            nc.tensor.matmul(
                                qt_ps[:],
                                wq_sb[:, dc, ts(m, P)],
                                xT_sb[g][:, dc, :],
                                start=(dc == 0), stop=(dc == ND - 1),
                            )
                        if k == 3:
                            nc.vector.tensor_scalar_add(
                                QT[:, m, ts(g, IT)], qt_ps[:],
                                bq_sb[:, ds(m, 1)])
                    return emit
                return [unit(k) for k in range(4)]

            def den_chain(ig, m, oe_sb, oo_sb, den2):
                """Normalize heads 2m, 2m+1 of i-group ig from their SBUF
                evacuations (o rows in oe_sb/oo_sb, denominators in den2)."""
                recip2 = den_pool.tile([1, 2, IT], F32)
                nc.vector.reciprocal_approx_fast(
                    recip2[:].rearrange("p a b -> p (a b)"),
                    den2[:].rearrange("p a b -> p (a b)"))
                recip2b = den_pool.tile([1, 2, IT], MMDT)
                nc.vector.tensor_copy(
                    recip2b[:].rearrange("p a b -> p (a b)"),
                    recip2[:].rearrange("p a b -> p (a b)"))
                bc_ps = ps_c.tile([P, IT], F32, tag="m")
                nc.tensor.matmul(bc_ps[ds(0, KS), :], ones64[:],
                                 recip2b[:, 0, :], start=True, stop=True)
                nc.tensor.matmul(bc_ps[ds(KS, KS), :], ones64[:],
                                 recip2b[:, 1, :], start=True, stop=True,
                                 tile_position=(0, KS))
                nc.vector.tensor_mul(
                    OT[ds(0, KS), m, ts(ig, IT)], oe_sb[:],
                    bc_ps[ds(0, KS), :])
                nc.vector.tensor_mul(
                    OT[ds(KS, KS), m, ts(ig, IT)], oo_sb[:],
                    bc_ps[ds(KS, KS), :])

            # ---- flat software pipeline over (ig, pair, jc) ----
            # The two heads of a pair occupy row groups 0-63 / 64-127, so
            # their score matmuls run CONCURRENTLY in the PE array (row
            # tiling) and one Exp covers both heads' scores.
            o_tiles = {}
            fill01, fill23 = [], []

            def build_fillers(ig):
                f01, f23 = [], []
                if ig == 0:
                    f01 = qt_units(1, 0) + qt_units(1, 1)
                else:
                    f01 = [wo_unit(4 * (ig - 1) + itl, ncol)
                           for itl in range(NI) for ncol in range(NO)]
                    if ig < NI - 1:
                        f23 = qt_units(ig + 1, 0) + qt_units(ig + 1, 1)
                return f01, f23

            def emit_av(pig, pm, pjc, ppt):
                for par in range(2):
                    key = (pig, 2 * pm + par)
                    if key not in o_tiles:
                        o_tiles[key] = ps_o.tile([KS + 1, IT], F32,
                                                 name="o_ps", tag="o")
                    nc.tensor.matmul(
                        o_tiles[key][:], V_aug[:, 2 * pm + par, pjc, :],
                        ppt[:, par, :],
                        start=(pjc == 0), stop=(pjc == NJ - 1),
                    )
                if pjc == NJ - 1:
                    # evacuate both heads to SBUF (fast PSUM release);
                    # the pair normalization chain is deferred so its
                    # broadcast matmul never blocks the PE on the
                    # reciprocal latency
                    den2 = den_pool.tile([1, 2, IT], F32)
                    evs = []
                    for par in range(2):
                        o_full = o_tiles.pop((pig, 2 * pm + par))
                        o_sb = den_pool.tile([KS, IT], F32, name="o_evac",
                                             tag=f"oev{par}", bufs=2)
                        nc.vector.tensor_copy(o_sb[:], o_full[ds(0, KS), :])
                        nc.vector.tensor_copy(den2[:, par, :],
                                              o_full[ds(KS, 1), :])
                        evs.append(o_sb)
                    chain_q.append([2, (pig, pm, evs[0], evs[1], den2)])

            pending = []  # (ig, m, jc, pt), AV emitted at depth 2
            chain_q = []  # deferred pair normalization chains
            for ig in range(NI):
                for f in fill01 + fill23:
                    f()
                fill01, fill23 = build_fillers(ig)
                for pr in range(NM):
                    fillers = fill01 if pr == 0 else fill23
                    for jc in range(NJ):
                        sP = ps_s.tile([P, NJJ, IT], F32, tag="s")
                        nc.tensor.matmul(
                            sP[:, 0, :],
                            KT[ds(0, KS), pr, ts(jc, P)],
                            QT[ds(0, KS), pr, ts(ig, IT)],
                            start=True, stop=True,
                        )
                        nc.tensor.matmul(
                            sP[:, 1, :],
                            KT[ds(KS, KS), pr, ts(jc, P)],
                            QT[ds(KS, KS), pr, ts(ig, IT)],
                            start=True, stop=True,
                        )
                        if len(pending) >= 3:
                            emit_av(*pending.pop(0))
                        if chain_q:
                            chain_q[0][0] -= 1
                            if chain_q[0][0] <= 0:
                                den_chain(*chain_q.pop(0)[1])
                        # Wo fillers read OT, written by the deferred chain
                        # (emitted by jc==3 of the next pair) -> pump late
                        if jc % 2 == 1 and jc >= 3 and fillers:
                            fillers.pop(0)()
                            if jc == NJ - 1 and fillers:
                                fillers.pop(0)()
                        pt = probs_pool.tile([P, NJJ, IT], MMDT)
                        nc.scalar.activation(
                            pt[:].rearrange("p a b -> p (a b)"),
                            sP[:].rearrange("p a b -> p (a b)"),
                            EXP, scale=0.125)
                        pending.append((ig, pr, jc, pt))
            # drain; junk matmuls keep the PE warm through the last chain
            wu_ps2 = ps_s.tile([P, NJJ, IT], F32, name="wu_ps2", tag="s")
            emit_av(*pending.pop(0))
            for _ in range(4):
                nc.tensor.matmul(wu_ps2[:, 0, :], wu_l[:], wu_r[:],
                                 start=True, stop=True)
            emit_av(*pending.pop(0))
            for _ in range(4):
                nc.tensor.matmul(wu_ps2[:, 0, :], wu_l[:], wu_r[:],
                                 start=True, stop=True)
            emit_av(*pending.pop(0))
            while chain_q:
                den_chain(*chain_q.pop(0)[1])
            for _ in range(12):
                nc.tensor.matmul(wu_ps2[:, 0, :], wu_l[:], wu_r[:],
                                 start=True, stop=True)
            nc.vector.tensor_copy(wu_sb[:], wu_ps2[ds(0, 1), 0, ds(0, 1)])
            for f in fill01 + fill23:
                f()

            # tail: Wo of the last i-group
            for itl in range(NI):
                it = 4 * (NI - 1) + itl
                for ncol in range(NO):
                    wo_unit(it, ncol)()


def _build_program():
    nc = bacc.Bacc("TRN2", target_bir_lowering=False, debug=False,
                   num_devices=NCORES)
    xT = nc.dram_tensor("xT", (D, S), MMDT, kind="ExternalInput").ap()
    wq = nc.dram_tensor("wq", (D, GW), MMDT, kind="ExternalInput").ap()
    wk = nc.dram_tensor("wk", (D, GW), MMDT, kind="ExternalInput").ap()
    wv = nc.dram_tensor("wv", (D, GW), MMDT, kind="ExternalInput").ap()
    wo = nc.dram_tensor("wo", (GW, D), MMDT, kind="ExternalInput").ap()
    bq = nc.dram_tensor("bq", (GW,), F32, kind="ExternalInput").ap()
    bk = nc.dram_tensor("bk", (GW,), F32, kind="ExternalInput").ap()
    bv = nc.dram_tensor("bv", (GW,), F32, kind="ExternalInput").ap()
    out = nc.dram_tensor("out", (S, D), MMDT, kind="ExternalOutput").ap()
    with tile.TileContext(nc) as tc:
        _mha_core(tc, out, xT, wq, wk, wv, wo, bq, bk, bv)
    nc.compile()
    return nc


_program = None


def _get_program():
    global _program
    if _program is None:
        _program = _build_program()
    return _program


def make_in_maps(x, Wq, bq, Wk, bk, Wv, bv, Wo, bo):
    in_maps = []
    f = np.float32
    bf = mybir.dt.np(MMDT)
    for c in range(NCORES):
        b, g = divmod(c, 4)
        sl = slice(g * GW, (g + 1) * GW)
        in_maps.append({
            "xT": np.ascontiguousarray(x[b].T).astype(bf),
            "wq": np.ascontiguousarray(Wq[:, sl]).astype(bf),
            "wk": np.ascontiguousarray(Wk[:, sl]).astype(bf),
            "wv": np.ascontiguousarray(Wv[:, sl]).astype(bf),
            "wo": np.ascontiguousarray(Wo[sl, :]).astype(bf),
            "bq": np.ascontiguousarray(bq[sl], dtype=f),
            "bk": np.ascontiguousarray(bk[sl], dtype=f),
            "bv": np.ascontiguousarray(bv[sl], dtype=f),
        })
    return in_maps


def run(inputs, trace=False, tmpdir=None, **kw):
    nc = _get_program()
    in_maps = make_in_maps(**inputs)
    res = run_bass_kernel_spmd(nc, in_maps, core_ids=list(range(NCORES)),
                               trace=trace, tmpdir=tmpdir, **kw)
    bo = inputs["bo"].astype(np.float32)
    parts = [np.asarray(res.results[c]["out"], dtype=np.float32)
             for c in range(NCORES)]
    y = np.stack(
        [parts[4 * b] + parts[4 * b + 1] + parts[4 * b + 2] + parts[4 * b + 3] + bo
         for b in range(B)], axis=0)
    return y.astype(np.float32), res


def kernel(**inputs):
    y, _ = run(inputs, trace=False)
    return y


# revision 23
# speedup vs baseline: 1.0321x; 1.0041x over previous
"""Multi-head attention (B=2, S=2048, D=1024, H=16, K=64) on 8 TRN2 cores.

Sharding: core c -> batch b=c//4, head-group g=c%4 (4 heads, 256-wide slice
of Wq/Wk/Wv columns and Wo rows).  Each core computes a partial (2048, 1024)
output in bf16; host sums groups of 4 cores in f32 and adds bo.

Per-core layout (all transposed so no on-chip transposes are needed):
  - host supplies xT = x[b].T  (D, S), bf16
  - Q^T, K^T computed as [gw_col, S] via lhsT=W chunk, rhs=xT chunk
  - scores^T[j, i] via lhsT=K^T chunk, rhs=Q^T into double-buffered 2-bank
    PSUM tiles; one ScalarE Exp covers 1024 elements
  - softmax denominator via a ones column appended to V (V_aug); probs are
    exp(scores/8) with no max subtraction (scores ~N(0,1), no overflow)
  - O^T = V_aug^T @ probs^T; division by the denominator happens on VectorE
    with head-pair batching (one reciprocal per pair, broadcast matmuls to
    the two column groups of one PSUM bank)

Scheduling: the attention inner loop alone cannot keep TensorE busy (the
Exp on ScalarE is the per-iteration rate limiter), and PE micro-idles make
the HAM clock-gate re-throttle the array to 1.2 GHz.  So the Wo matmuls of
the previous i-group and the Q^T projection of the next i-group are pumped
into the PE queue as filler work between score groups, keeping the PE
saturated and warm.  AV matmuls are emitted one score-group behind so the
PE never head-of-line blocks on the Exp.

All matmul operands are bf16 (PSUM accumulation stays fp32).
"""

import os
import sys
from contextlib import ExitStack

import numpy as np

if "/opt/trn_rl_repo" not in sys.path:
    sys.path.insert(0, "/opt/trn_rl_repo")

import concourse.bass as bass
import concourse.mybir as mybir
import concourse.tile as tile
from concourse import bacc
from concourse.bass import ds, ts
from concourse.bass_utils import run_bass_kernel_spmd

B, S, D = 2, 2048, 1024
H, KS = 16, 64
NCORES = 8
HPC = H // 4          # 4 heads per core
GW = HPC * KS         # 256-wide head-group slice
P = 128
ND = D // P           # 8 contraction chunks over d_model
NM = GW // P          # 2 col chunks of the group slice
NI = 4                # i-groups
IT = S // NI          # 512 rows per i-group
NJ = S // P           # 16 j-chunks
NJJ = 2               # j-chunks per Exp batch ([128,1024] ACT, 2 PSUM banks)
NG = NJ // NJJ        # score groups per head
NO = D // 512         # 2 out-col groups for Wo

F32 = mybir.dt.float32
BF16 = mybir.dt.bfloat16
MMDT = BF16
EXP = mybir.ActivationFunctionType.Exp


def _mha_core(tc, out, xT, wq, wk, wv, wo, bq, bk, bv):
    nc = tc.nc
    with ExitStack() as ctx:
        cp = ctx.enter_context(tc.tile_pool(name="const", bufs=1))
        probs_pool = ctx.enter_context(tc.tile_pool(name="probs", bufs=5))
        out_pool = ctx.enter_context(tc.tile_pool(name="outsb", bufs=3))
        den_pool = ctx.enter_context(tc.tile_pool(name="den", bufs=2))

        # ---- ACT table preload: tiny exp before anything else on ScalarE ----
        warm = cp.tile([1, 16], F32)
        nc.vector.memset(warm[:], 0.0)
        nc.scalar.activation(warm[:], warm[:], EXP)

        # HAM warmup operands: junk matmuls keep the PE busy while DMAs
        # land (and during the tail den-chain) so real matmuls run at
        # 2.4 GHz instead of the cold 1.2 GHz
        wu_l = cp.tile([P, P], MMDT)
        wu_r = cp.tile([P, IT], MMDT)
        nc.vector.memset(wu_l[:], 0.0)
        nc.vector.memset(wu_r[:], 0.0)

        # ---- inputs to SBUF; sync ring carries what's needed first ----
        wk_sb = cp.tile([P, ND, GW], MMDT)
        nc.sync.dma_start(wk_sb[:], wk.rearrange("(nd p) n -> p nd n", p=P))
        xT_sb = []
        for c4 in range(4):
            xc = cp.tile([P, ND, IT], MMDT, name=f"xc{c4}")
            nc.sync.dma_start(
                xc[:], xT[:, ts(c4, IT)].rearrange("(nd p) s -> p nd s", p=P))
            xT_sb.append(xc)
        wq_sb = cp.tile([P, ND, GW], MMDT)
        nc.sync.dma_start(wq_sb[:], wq.rearrange("(nd p) n -> p nd n", p=P))
        wv_sb = cp.tile([P, ND, GW], MMDT)
        wo_sb = cp.tile([P, NM, D], MMDT)
        nc.scalar.dma_start(wv_sb[:], wv.rearrange("(nd p) n -> p nd n", p=P))
        nc.scalar.dma_start(wo_sb[:], wo.rearrange("(nm p) n -> p nm n", p=P))
        bq_sb = cp.tile([P, NM], F32)
        bk_sb = cp.tile([P, NM], F32)
        nc.scalar.dma_start(bq_sb[:], bq.rearrange("(m p) -> p m", p=P))
        nc.scalar.dma_start(bk_sb[:], bk.rearrange("(m p) -> p m", p=P))
        bv_bc = cp.tile([P, GW], F32)
        nc.scalar.dma_start(bv_bc[:], bv.partition_broadcast(P))

        QT = cp.tile([P, NM, S], MMDT)
        KT = cp.tile([P, NM, S], MMDT)
        OT = cp.tile([P, NM, S], MMDT)
        # V_aug[:, h, jt, 0:64] = V rows, [:, h, jt, 64] = 1.0 (denominator col)
        V_aug = cp.tile([P, HPC, NJ, KS + 1], MMDT)
        nc.vector.memset(
            V_aug[:, :, :, ds(KS, 1)].rearrange("p h j o -> p (h j o)"), 1.0)

        # ones row for the denominator broadcast matmul (1/den -> 64 rows)
        ones64 = cp.tile([1, KS], MMDT)
        nc.vector.memset(ones64[:], 1.0)

        # ---- projections: K^T (ig-major, chasing the xT DMAs), V, Q^T(0) ----
        with tc.tile_pool(name="ps_acc", bufs=4, space="PSUM") as ps_acc:
            wu_ps = ps_acc.tile([P, IT], F32, tag="acc")
            for _ in range(16):
                nc.tensor.matmul(wu_ps[:], wu_l[:], wu_r[:],
                                 start=True, stop=True)
            wu_sb = cp.tile([1, 1], F32)
            nc.vector.tensor_copy(wu_sb[:], wu_ps[ds(0, 1), ds(0, 1)])
            for c4 in range(NI):
                for m in range(NM):
                    kt_ps = ps_acc.tile([P, IT], F32, tag="acc")
                    for dc in range(ND):
                        nc.tensor.matmul(
                            kt_ps[:],
                            wk_sb[:, dc, ts(m, P)],
                            xT_sb[c4][:, dc, :],
                            start=(dc == 0), stop=(dc == ND - 1),
                        )
                    nc.vector.tensor_scalar_add(
                        KT[:, m, ts(c4, IT)], kt_ps[:], bk_sb[:, ds(m, 1)])
                for jt in range(4 * c4, 4 * c4 + 4):
                    v_ps = ps_acc.tile([P, IT], F32, tag="acc")
                    for dc in range(ND):
                        nc.tensor.matmul(
                            v_ps[:, 0:GW],
                            xT_sb[jt // 4][:, dc, ts(jt % 4, P)],
                            wv_sb[:, dc, :],
                            start=(dc == 0), stop=(dc == ND - 1),
                        )
                    nc.vector.tensor_add(
                        V_aug[:, :, jt, 0:KS],
                        v_ps[:, 0:GW].rearrange("p (h k) -> p h k", h=HPC),
                        bv_bc[:].rearrange("p (h k) -> p h k", h=HPC),
                    )

            for m in range(NM):
                qt_ps = ps_acc.tile([P, IT], F32, tag="acc")
                for dc in range(ND):
                    nc.tensor.matmul(
                        qt_ps[:],
                        wq_sb[:, dc, ts(m, P)],
                        xT_sb[0][:, dc, :],
                        start=(dc == 0), stop=(dc == ND - 1),
                    )
                nc.vector.tensor_scalar_add(
                    QT[:, m, ts(0, IT)], qt_ps[:], bq_sb[:, ds(m, 1)])

        # ---- attention + interleaved Wo / Q^T-projection filler ----
        # PSUM budget: s2 = 2x2, o_ps = 2, misc(w/qt/bc) = 2 shared slots
        with tc.tile_pool(name="ps_s", bufs=2, space="PSUM") as ps_s, \
             tc.tile_pool(name="ps_o", bufs=2, space="PSUM") as ps_o, \
             tc.tile_pool(name="ps_m", bufs=2, space="PSUM") as ps_m:
            ps_c = ps_m

            stage, stage_left = {}, {}

            def wo_unit(it, ncol):
                def emit():
                    g = it // NI
                    if g not in stage and g != NI - 1:
                        stage[g] = out_pool.tile([P, NI, D], MMDT,
                                                 name="ostage", tag="ost",
                                                 bufs=2)
                        stage_left[g] = NI * NO
                    w_ps = ps_m.tile([P, 512], F32, tag="m")
                    for hc in range(NM):
                        nc.tensor.matmul(
                            w_ps[:],
                            OT[:, hc, ts(it, P)],
                            wo_sb[:, hc, ts(ncol, 512)],
                            start=(hc == 0), stop=(hc == NM - 1),
                        )
                    if g == NI - 1:
                        o_sb = out_pool.tile([P, 512], MMDT)
                        if (it + ncol) % 2 == 0:
                            nc.vector.tensor_copy(o_sb[:], w_ps[:])
                            nc.sync.dma_start(
                                out[ts(it, P), ts(ncol, 512)], o_sb[:])
                        else:
                            nc.scalar.copy(o_sb[:], w_ps[:])
                            nc.scalar.dma_start(
                                out[ts(it, P), ts(ncol, 512)], o_sb[:])
                        return
                    st = stage[g]
                    nc.vector.tensor_copy(st[:, it % NI, ts(ncol, 512)],
                                          w_ps[:])
                    stage_left[g] -= 1
                    if stage_left[g] == 0:
                        eng = nc.sync if g % 2 == 0 else nc.scalar
                        eng.dma_start(
                            out[ts(g, 4 * P), :].rearrange(
                                "(itl p) d -> p itl d", p=P),
                            st[:])
                        del stage[g]
                return emit

            def qt_units(g, m):
                """Q^T projection of i-group g, col chunk m, as 4 filler
                units of 2 accumulating matmuls each."""
                state = {}

                def unit(k):
                    def emit():
                        if k == 0:
                            state["ps"] = ps_m.tile([P, IT], F32,
                                                    name="qt_fill_ps", tag="m")
                        qt_ps = state["ps"]
                        for dc in (2 * k, 2 * k + 1):
                            nc.tensor.matmul(
                                qt_ps[:],
                                wq_sb[:, dc, ts(m, P)],
                                xT_sb[g][:, dc, :],
                                start=(dc == 0), stop=(dc == ND - 1),
                            )
                        if k == 3:
                            nc.vector.tensor_scalar_add(
                                QT[:, m, ts(g, IT)], qt_ps[:],
                                bq_sb[:, ds(m, 1)])
                    return emit
                return [unit(k) for k in range(4)]

            def den_chain(ig, m, oe_sb, oo_sb, den2):
                """Normalize heads 2m, 2m+1 of i-group ig from their SBUF
                evacuations (o rows in oe_sb/oo_sb, denominators in den2)."""
                recip2 = den_pool.tile([1, 2, IT], F32)
                nc.vector.reciprocal_approx_fast(
                    recip2[:].rearrange("p a b -> p (a b)"),
                    den2[:].rearrange("p a b -> p (a b)"))
                recip2b = den_pool.tile([1, 2, IT], MMDT)
                nc.vector.tensor_copy(
                    recip2b[:].rearrange("p a b -> p (a b)"),
                    recip2[:].rearrange("p a b -> p (a b)"))
                bc_ps = ps_c.tile([P, IT], F32, tag="m")
                nc.tensor.matmul(bc_ps[ds(0, KS), :], ones64[:],
                                 recip2b[:, 0, :], start=True, stop=True)
                nc.tensor.matmul(bc_ps[ds(KS, KS), :], ones64[:],
                                 recip2b[:, 1, :], start=True, stop=True,
                                 tile_position=(0, KS))
                nc.vector.tensor_mul(
                    OT[ds(0, KS), m, ts(ig, IT)], oe_sb[:],
                    bc_ps[ds(0, KS), :])
                nc.vector.tensor_mul(
                    OT[ds(KS, KS), m, ts(ig, IT)], oo_sb[:],
                    bc_ps[ds(KS, KS), :])

            # ---- flat software pipeline over (ig, pair, jc) ----
            # The two heads of a pair occupy row groups 0-63 / 64-127, so
            # their score matmuls run CONCURRENTLY in the PE array (row
            # tiling) and one Exp covers both heads' scores.
            o_tiles = {}
            fill01, fill23 = [], []

            def build_fillers(ig):
                f01, f23 = [], []
                if ig == 0:
                    f01 = qt_units(1, 0) + qt_units(1, 1)
                else:
                    f01 = [wo_unit(4 * (ig - 1) + itl, ncol)
                           for itl in range(NI) for ncol in range(NO)]
                    if ig < NI - 1:
                        f23 = qt_units(ig + 1, 0) + qt_units(ig + 1, 1)
                return f01, f23

            def emit_av(pig, pm, pjc, ppt):
                for par in range(2):
                    key = (pig, 2 * pm + par)
                    if key not in o_tiles:
                        o_tiles[key] = ps_o.tile([KS + 1, IT], F32,
                                                 name="o_ps", tag="o")
                    nc.tensor.matmul(
                        o_tiles[key][:], V_aug[:, 2 * pm + par, pjc, :],
                        ppt[:, par, :],
                        start=(pjc == 0), stop=(pjc == NJ - 1),
                    )
                if pjc == NJ - 1:
                    # evacuate both heads to SBUF (fast PSUM release);
                    # the pair normalization chain is deferred so its
                    # broadcast matmul never blocks the PE on the
                    # reciprocal latency
                    den2 = den_pool.tile([1, 2, IT], F32)
                    evs = []
                    for par in range(2):
                        o_full = o_tiles.pop((pig, 2 * pm + par))
                        o_sb = den_pool.tile([KS, IT], F32, name="o_evac",
                                             tag=f"oev{par}", bufs=2)
                        nc.vector.tensor_copy(o_sb[:], o_full[ds(0, KS), :])
                        nc.vector.tensor_copy(den2[:, par, :],
                                              o_full[ds(KS, 1), :])
                        evs.append(o_sb)
                    chain_q.append([2, (pig, pm, evs[0], evs[1], den2)])

            pending = []  # (ig, m, jc, pt), AV emitted at depth 2
            chain_q = []  # deferred pair normalization chains
            for ig in range(NI):
                for f in fill01 + fill23:
                    f()
                fill01, fill23 = build_fillers(ig)
                for pr in range(NM):
                    fillers = fill01 if pr == 0 else fill23
                    for jc in range(NJ):
                        sP = ps_s.tile([P, NJJ, IT], F32, tag="s")
                        nc.tensor.matmul(
                            sP[:, 0, :],
                            KT[ds(0, KS), pr, ts(jc, P)],
                            QT[ds(0, KS), pr, ts(ig, IT)],
                            start=True, stop=True,
                        )
                        nc.tensor.matmul(
                            sP[:, 1, :],
                            KT[ds(KS, KS), pr, ts(jc, P)],
                            QT[ds(KS, KS), pr, ts(ig, IT)],
                            start=True, stop=True,
                        )
                        if len(pending) >= 3:
                            emit_av(*pending.pop(0))
                        if chain_q:
                            chain_q[0][0] -= 1
                            if chain_q[0][0] <= 0:
                                den_chain(*chain_q.pop(0)[1])
                        # Wo fillers read OT, written by the deferred chain
                        # (emitted by jc==3 of the next pair) -> pump late
                        if jc % 2 == 1 and jc >= 3 and fillers:
                            fillers.pop(0)()
                            if jc == NJ - 1 and fillers:
                                fillers.pop(0)()
                        pt = probs_pool.tile([P, NJJ, IT], MMDT)
                        nc.scalar.activation(
                            pt[:].rearrange("p a b -> p (a b)"),
                            sP[:].rearrange("p a b -> p (a b)"),
                            EXP, scale=0.125)
                        pending.append((ig, pr, jc, pt))
            # drain; junk matmuls keep the PE warm through the last chain
            wu_ps2 = ps_s.tile([P, NJJ, IT], F32, name="wu_ps2", tag="s")
            emit_av(*pending.pop(0))
            for _ in range(4):
                nc.tensor.matmul(wu_ps2[:, 0, :], wu_l[:], wu_r[:],
                                 start=True, stop=True)
            emit_av(*pending.pop(0))
            for _ in range(4):
                nc.tensor.matmul(wu_ps2[:, 0, :], wu_l[:], wu_r[:],
                                 start=True, stop=True)
            emit_av(*pending.pop(0))
            while chain_q:
                den_chain(*chain_q.pop(0)[1])
            for _ in range(12):
                nc.tensor.matmul(wu_ps2[:, 0, :], wu_l[:], wu_r[:],
                                 start=True, stop=True)
            nc.vector.tensor_copy(wu_sb[:], wu_ps2[ds(0, 1), 0, ds(0, 1)])
            for f in fill01 + fill23:
                f()

            # tail: Wo of the last i-group
            for itl in range(NI):
                it = 4 * (NI - 1) + itl
                for ncol in range(NO):
                    wo_unit(it, ncol)()


def _build_program():
    nc = bacc.Bacc("TRN2", target_bir_lowering=False, debug=False,
                   num_devices=NCORES)
    xT = nc.dram_tensor("xT", (D, S), MMDT, kind="ExternalInput").ap()
    wq = nc.dram_tensor("wq", (D, GW), MMDT, kind="ExternalInput").ap()
    wk = nc.dram_tensor("wk", (D, GW), MMDT, kind="ExternalInput").ap()
    wv = nc.dram_tensor("wv", (D, GW), MMDT, kind="ExternalInput").ap()
    wo = nc.dram_tensor("wo", (GW, D), MMDT, kind="ExternalInput").ap()
    bq = nc.dram_tensor("bq", (GW,), F32, kind="ExternalInput").ap()
    bk = nc.dram_tensor("bk", (GW,), F32, kind="ExternalInput").ap()
    bv = nc.dram_tensor("bv", (GW,), F32, kind="ExternalInput").ap()
    out = nc.dram_tensor("out", (S, D), MMDT, kind="ExternalOutput").ap()
    with tile.TileContext(nc) as tc:
        _mha_core(tc, out, xT, wq, wk, wv, wo, bq, bk, bv)
    nc.compile()
    return nc


_program = None


def _get_program():
    global _program
    if _program is None:
        _program = _build_program()
    return _program


def make_in_maps(x, Wq, bq, Wk, bk, Wv, bv, Wo, bo):
    in_maps = []
    f = np.float32
    bf = mybir.dt.np(MMDT)
    for c in range(NCORES):
        b, g = divmod(c, 4)
        sl = slice(g * GW, (g + 1) * GW)
        in_maps.append({
            "xT": np.ascontiguousarray(x[b].T).astype(bf),
            "wq": np.ascontiguousarray(Wq[:, sl]).astype(bf),
            "wk": np.ascontiguousarray(Wk[:, sl]).astype(bf),
            "wv": np.ascontiguousarray(Wv[:, sl]).astype(bf),
            "wo": np.ascontiguousarray(Wo[sl, :]).astype(bf),
            "bq": np.ascontiguousarray(bq[sl], dtype=f),
            "bk": np.ascontiguousarray(bk[sl], dtype=f),
            "bv": np.ascontiguousarray(bv[sl], dtype=f),
        })
    return in_maps


def run(inputs, trace=False, tmpdir=None, **kw):
    nc = _get_program()
    in_maps = make_in_maps(**inputs)
    res = run_bass_kernel_spmd(nc, in_maps, core_ids=list(range(NCORES)),
                               trace=trace, tmpdir=tmpdir, **kw)
    bo = inputs["bo"].astype(np.float32)
    parts = [np.asarray(res.results[c]["out"], dtype=np.float32)
             for c in range(NCORES)]
    y = np.stack(
        [parts[4 * b] + parts[4 * b + 1] + parts[4 * b + 2] + parts[4 * b + 3] + bo
         for b in range(B)], axis=0)
    return y.astype(np.float32), res


def kernel(**inputs):
    y, _ = run(inputs, trace=False)
    return y


# revision 24
# speedup vs baseline: 1.0434x; 1.0109x over previous
"""Multi-head attention (B=2, S=2048, D=1024, H=16, K=64) on 8 TRN2 cores.

Sharding: core c -> batch b=c//4, head-group g=c%4 (4 heads, 256-wide slice
of Wq/Wk/Wv columns and Wo rows).  Each core computes a partial (2048, 1024)
output in bf16; host sums groups of 4 cores in f32 and adds bo.

Per-core layout (all transposed so no on-chip transposes are needed):
  - host supplies xT = x[b].T  (D, S), bf16
  - Q^T, K^T computed as [gw_col, S] via lhsT=W chunk, rhs=xT chunk
  - scores^T[j, i] via lhsT=K^T chunk, rhs=Q^T into double-buffered 2-bank
    PSUM tiles; one ScalarE Exp covers 1024 elements
  - softmax denominator via a ones column appended to V (V_aug); probs are
    exp(scores/8) with no max subtraction (scores ~N(0,1), no overflow)
  - O^T = V_aug^T @ probs^T; division by the denominator happens on VectorE
    with head-pair batching (one reciprocal per pair, broadcast matmuls to
    the two column groups of one PSUM bank)

Scheduling: the attention inner loop alone cannot keep TensorE busy (the
Exp on ScalarE is the per-iteration rate limiter), and PE micro-idles make
the HAM clock-gate re-throttle the array to 1.2 GHz.  So the Wo matmuls of
the previous i-group and the Q^T projection of the next i-group are pumped
into the PE queue as filler work between score groups, keeping the PE
saturated and warm.  AV matmuls are emitted one score-group behind so the
PE never head-of-line blocks on the Exp.

All matmul operands are bf16 (PSUM accumulation stays fp32).
"""

import os
import sys
from contextlib import ExitStack

import numpy as np

if "/opt/trn_rl_repo" not in sys.path:
    sys.path.insert(0, "/opt/trn_rl_repo")

import concourse.bass as bass
import concourse.mybir as mybir
import concourse.tile as tile
from concourse import bacc
from concourse.bass import ds, ts
from concourse.bass_utils import run_bass_kernel_spmd

B, S, D = 2, 2048, 1024
H, KS = 16, 64
NCORES = 8
HPC = H // 4          # 4 heads per core
GW = HPC * KS         # 256-wide head-group slice
P = 128
ND = D // P           # 8 contraction chunks over d_model
NM = GW // P          # 2 col chunks of the group slice
NI = 4                # i-groups
IT = S // NI          # 512 rows per i-group
NJ = S // P           # 16 j-chunks
NJJ = 2               # j-chunks per Exp batch ([128,1024] ACT, 2 PSUM banks)
NG = NJ // NJJ        # score groups per head
NO = D // 512         # 2 out-col groups for Wo

F32 = mybir.dt.float32
BF16 = mybir.dt.bfloat16
MMDT = BF16
EXP = mybir.ActivationFunctionType.Exp


def _mha_core(tc, out, xT, wq, wk, wv, wo, bq, bk, bv):
    nc = tc.nc
    with ExitStack() as ctx:
        cp = ctx.enter_context(tc.tile_pool(name="const", bufs=1))
        probs_pool = ctx.enter_context(tc.tile_pool(name="probs", bufs=5))
        out_pool = ctx.enter_context(tc.tile_pool(name="outsb", bufs=3))
        den_pool = ctx.enter_context(tc.tile_pool(name="den", bufs=2))

        # ---- ACT table preload: tiny exp before anything else on ScalarE ----
        warm = cp.tile([1, 16], F32)
        nc.vector.memset(warm[:], 0.0)
        nc.scalar.activation(warm[:], warm[:], EXP)

        # HAM warmup operands: junk matmuls keep the PE busy while DMAs
        # land (and during the tail den-chain) so real matmuls run at
        # 2.4 GHz instead of the cold 1.2 GHz
        wu_l = cp.tile([P, P], MMDT)
        wu_r = cp.tile([P, IT], MMDT)
        nc.vector.memset(wu_l[:], 0.0)
        nc.vector.memset(wu_r[:], 0.0)

        # ---- inputs to SBUF; sync ring carries what's needed first ----
        wk_sb = cp.tile([P, ND, GW], MMDT)
        nc.sync.dma_start(wk_sb[:], wk.rearrange("(nd p) n -> p nd n", p=P))
        xT_sb = []
        for c4 in range(4):
            xc = cp.tile([P, ND, IT], MMDT, name=f"xc{c4}")
            nc.sync.dma_start(
                xc[:], xT[:, ts(c4, IT)].rearrange("(nd p) s -> p nd s", p=P))
            xT_sb.append(xc)
        wq_sb = cp.tile([P, ND, GW], MMDT)
        nc.sync.dma_start(wq_sb[:], wq.rearrange("(nd p) n -> p nd n", p=P))
        wv_sb = cp.tile([P, ND, GW], MMDT)
        wo_sb = cp.tile([P, NM, D], MMDT)
        nc.scalar.dma_start(wv_sb[:], wv.rearrange("(nd p) n -> p nd n", p=P))
        nc.scalar.dma_start(wo_sb[:], wo.rearrange("(nm p) n -> p nm n", p=P))
        bq_sb = cp.tile([P, NM], F32)
        bk_sb = cp.tile([P, NM], F32)
        nc.scalar.dma_start(bq_sb[:], bq.rearrange("(m p) -> p m", p=P))
        nc.scalar.dma_start(bk_sb[:], bk.rearrange("(m p) -> p m", p=P))
        bv_bc = cp.tile([P, GW], F32)
        nc.scalar.dma_start(bv_bc[:], bv.partition_broadcast(P))

        QT = cp.tile([P, NM, S], MMDT)
        KT = cp.tile([P, NM, S], MMDT)
        OT = cp.tile([P, NM, S], MMDT)
        # V_aug[:, h, jt, 0:64] = V rows, [:, h, jt, 64] = 1.0 (denominator col)
        V_aug = cp.tile([P, HPC, NJ, KS + 1], MMDT)
        nc.vector.memset(
            V_aug[:, :, :, ds(KS, 1)].rearrange("p h j o -> p (h j o)"), 1.0)

        # ones row for the denominator broadcast matmul (1/den -> 64 rows)
        ones64 = cp.tile([1, KS], MMDT)
        nc.vector.memset(ones64[:], 1.0)

        # ---- projections: K^T (ig-major, chasing the xT DMAs), V, Q^T(0) ----
        with tc.tile_pool(name="ps_acc", bufs=4, space="PSUM") as ps_acc:
            wu_ps = ps_acc.tile([P, IT], F32, tag="acc")
            for _ in range(16):
                nc.tensor.matmul(wu_ps[:], wu_l[:], wu_r[:],
                                 start=True, stop=True)
            wu_sb = cp.tile([1, 1], F32)
            nc.vector.tensor_copy(wu_sb[:], wu_ps[ds(0, 1), ds(0, 1)])
            for c4 in range(NI):
                for m in range(NM):
                    kt_ps = ps_acc.tile([P, IT], F32, tag="acc")
                    for dc in range(ND):
                        nc.tensor.matmul(
                            kt_ps[:],
                            wk_sb[:, dc, ts(m, P)],
                            xT_sb[c4][:, dc, :],
                            start=(dc == 0), stop=(dc == ND - 1),
                        )
                    nc.vector.tensor_scalar_add(
                        KT[:, m, ts(c4, IT)], kt_ps[:], bk_sb[:, ds(m, 1)])
                for jt in range(4 * c4, 4 * c4 + 4):
                    v_ps = ps_acc.tile([P, IT], F32, tag="acc")
                    for dc in range(ND):
                        nc.tensor.matmul(
                            v_ps[:, 0:GW],
                            xT_sb[jt // 4][:, dc, ts(jt % 4, P)],
                            wv_sb[:, dc, :],
                            start=(dc == 0), stop=(dc == ND - 1),
                        )
                    nc.vector.tensor_add(
                        V_aug[:, :, jt, 0:KS],
                        v_ps[:, 0:GW].rearrange("p (h k) -> p h k", h=HPC),
                        bv_bc[:].rearrange("p (h k) -> p h k", h=HPC),
                    )

            for m in range(NM):
                qt_ps = ps_acc.tile([P, IT], F32, tag="acc")
                for dc in range(ND):
                    nc.tensor.matmul(
                        qt_ps[:],
                        wq_sb[:, dc, ts(m, P)],
                        xT_sb[0][:, dc, :],
                        start=(dc == 0), stop=(dc == ND - 1),
                    )
                nc.vector.tensor_scalar_add(
                    QT[:, m, ts(0, IT)], qt_ps[:], bq_sb[:, ds(m, 1)])

        # ---- attention + interleaved Wo / Q^T-projection filler ----
        # PSUM budget: s2 = 2x2, o_ps = 2, misc(w/qt/bc) = 2 shared slots
        with tc.tile_pool(name="ps_s", bufs=2, space="PSUM") as ps_s, \
             tc.tile_pool(name="ps_o", bufs=2, space="PSUM") as ps_o, \
             tc.tile_pool(name="ps_m", bufs=2, space="PSUM") as ps_m:
            ps_c = ps_m

            stage, stage_left = {}, {}

            def wo_unit(it, ncol):
                def emit():
                    g = it // NI
                    if g not in stage and g != NI - 1:
                        stage[g] = out_pool.tile([P, NI, D], MMDT,
                                                 name="ostage", tag="ost",
                                                 bufs=2)
                        stage_left[g] = NI * NO
                    w_ps = ps_m.tile([P, 512], F32, tag="m")
                    for hc in range(NM):
                        nc.tensor.matmul(
                            w_ps[:],
                            OT[:, hc, ts(it, P)],
                            wo_sb[:, hc, ts(ncol, 512)],
                            start=(hc == 0), stop=(hc == NM - 1),
                        )
                    if g == NI - 1:
                        o_sb = out_pool.tile([P, 512], MMDT)
                        r = (it + ncol) % 3
                        if r == 1:
                            nc.scalar.copy(o_sb[:], w_ps[:])
                            nc.scalar.dma_start(
                                out[ts(it, P), ts(ncol, 512)], o_sb[:])
                        else:
                            nc.vector.tensor_copy(o_sb[:], w_ps[:])
                            eng = nc.sync if r == 0 else nc.gpsimd
                            eng.dma_start(
                                out[ts(it, P), ts(ncol, 512)], o_sb[:])
                        return
                    st = stage[g]
                    nc.vector.tensor_copy(st[:, it % NI, ts(ncol, 512)],
                                          w_ps[:])
                    stage_left[g] -= 1
                    if stage_left[g] == 0:
                        eng = nc.sync if g % 2 == 0 else nc.scalar
                        eng.dma_start(
                            out[ts(g, 4 * P), :].rearrange(
                                "(itl p) d -> p itl d", p=P),
                            st[:])
                        del stage[g]
                return emit

            def qt_units(g, m):
                """Q^T projection of i-group g, col chunk m, as 4 filler
                units of 2 accumulating matmuls each."""
                state = {}

                def unit(k):
                    def emit():
                        if k == 0:
                            state["ps"] = ps_m.tile([P, IT], F32,
                                                    name="qt_fill_ps", tag="m")
                        qt_ps = state["ps"]
                        for dc in (2 * k, 2 * k + 1):
                            nc.tensor.matmul(
                                qt_ps[:],
                                wq_sb[:, dc, ts(m, P)],
                                xT_sb[g][:, dc, :],
                                start=(dc == 0), stop=(dc == ND - 1),
                            )
                        if k == 3:
                            nc.vector.tensor_scalar_add(
                                QT[:, m, ts(g, IT)], qt_ps[:],
                                bq_sb[:, ds(m, 1)])
                    return emit
                return [unit(k) for k in range(4)]

            def den_chain(ig, m, oe_sb, oo_sb, den2):
                """Normalize heads 2m, 2m+1 of i-group ig from their SBUF
                evacuations (o rows in oe_sb/oo_sb, denominators in den2)."""
                recip2 = den_pool.tile([1, 2, IT], F32)
                nc.vector.reciprocal_approx_fast(
                    recip2[:].rearrange("p a b -> p (a b)"),
                    den2[:].rearrange("p a b -> p (a b)"))
                recip2b = den_pool.tile([1, 2, IT], MMDT)
                nc.vector.tensor_copy(
                    recip2b[:].rearrange("p a b -> p (a b)"),
                    recip2[:].rearrange("p a b -> p (a b)"))
                bc_ps = ps_c.tile([P, IT], F32, tag="m")
                nc.tensor.matmul(bc_ps[ds(0, KS), :], ones64[:],
                                 recip2b[:, 0, :], start=True, stop=True)
                nc.tensor.matmul(bc_ps[ds(KS, KS), :], ones64[:],
                                 recip2b[:, 1, :], start=True, stop=True,
                                 tile_position=(0, KS))
                nc.vector.tensor_mul(
                    OT[ds(0, KS), m, ts(ig, IT)], oe_sb[:],
                    bc_ps[ds(0, KS), :])
                nc.vector.tensor_mul(
                    OT[ds(KS, KS), m, ts(ig, IT)], oo_sb[:],
                    bc_ps[ds(KS, KS), :])

            # ---- flat software pipeline over (ig, pair, jc) ----
            # The two heads of a pair occupy row groups 0-63 / 64-127, so
            # their score matmuls run CONCURRENTLY in the PE array (row
            # tiling) and one Exp covers both heads' scores.
            o_tiles = {}
            fill01, fill23 = [], []

            def build_fillers(ig):
                f01, f23 = [], []
                if ig == 0:
                    f01 = qt_units(1, 0) + qt_units(1, 1)
                else:
                    f01 = [wo_unit(4 * (ig - 1) + itl, ncol)
                           for itl in range(NI) for ncol in range(NO)]
                    if ig < NI - 1:
                        f23 = qt_units(ig + 1, 0) + qt_units(ig + 1, 1)
                return f01, f23

            def emit_av(pig, pm, pjc, ppt):
                for par in range(2):
                    key = (pig, 2 * pm + par)
                    if key not in o_tiles:
                        o_tiles[key] = ps_o.tile([KS + 1, IT], F32,
                                                 name="o_ps", tag="o")
                    nc.tensor.matmul(
                        o_tiles[key][:], V_aug[:, 2 * pm + par, pjc, :],
                        ppt[:, par, :],
                        start=(pjc == 0), stop=(pjc == NJ - 1),
                    )
                if pjc == NJ - 1:
                    # evacuate both heads to SBUF (fast PSUM release);
                    # the pair normalization chain is deferred so its
                    # broadcast matmul never blocks the PE on the
                    # reciprocal latency
                    den2 = den_pool.tile([1, 2, IT], F32)
                    evs = []
                    for par in range(2):
                        o_full = o_tiles.pop((pig, 2 * pm + par))
                        o_sb = den_pool.tile([KS, IT], F32, name="o_evac",
                                             tag=f"oev{par}", bufs=2)
                        nc.vector.tensor_copy(o_sb[:], o_full[ds(0, KS), :])
                        nc.vector.tensor_copy(den2[:, par, :],
                                              o_full[ds(KS, 1), :])
                        evs.append(o_sb)
                    chain_q.append([2, (pig, pm, evs[0], evs[1], den2)])

            pending = []  # (ig, m, jc, pt), AV emitted at depth 2
            chain_q = []  # deferred pair normalization chains
            for ig in range(NI):
                for f in fill01 + fill23:
                    f()
                fill01, fill23 = build_fillers(ig)
                for pr in range(NM):
                    fillers = fill01 if pr == 0 else fill23
                    for jc in range(NJ):
                        sP = ps_s.tile([P, NJJ, IT], F32, tag="s")
                        nc.tensor.matmul(
                            sP[:, 0, :],
                            KT[ds(0, KS), pr, ts(jc, P)],
                            QT[ds(0, KS), pr, ts(ig, IT)],
                            start=True, stop=True,
                        )
                        nc.tensor.matmul(
                            sP[:, 1, :],
                            KT[ds(KS, KS), pr, ts(jc, P)],
                            QT[ds(KS, KS), pr, ts(ig, IT)],
                            start=True, stop=True,
                        )
                        if len(pending) >= 3:
                            emit_av(*pending.pop(0))
                        if chain_q:
                            chain_q[0][0] -= 1
                            if chain_q[0][0] <= 0:
                                den_chain(*chain_q.pop(0)[1])
                        # Wo fillers read OT, written by the deferred chain
                        # (emitted by jc==3 of the next pair) -> pump late;
                        # QT fillers have no such hazard -> front-load them
                        # so QT is ready well before the next i-group
                        if pr == 0:
                            if jc % 2 == 1 and jc >= 3 and fillers:
                                fillers.pop(0)()
                                if jc == NJ - 1 and fillers:
                                    fillers.pop(0)()
                        elif jc % 2 == 1 and jc < 8:
                            for _ in range(2):
                                if fillers:
                                    fillers.pop(0)()
                        pt = probs_pool.tile([P, NJJ, IT], MMDT)
                        nc.scalar.activation(
                            pt[:].rearrange("p a b -> p (a b)"),
                            sP[:].rearrange("p a b -> p (a b)"),
                            EXP, scale=0.125)
                        pending.append((ig, pr, jc, pt))
            # drain; junk matmuls keep the PE warm through the last chain
            wu_ps2 = ps_s.tile([P, NJJ, IT], F32, name="wu_ps2", tag="s")
            emit_av(*pending.pop(0))
            for _ in range(4):
                nc.tensor.matmul(wu_ps2[:, 0, :], wu_l[:], wu_r[:],
                                 start=True, stop=True)
            emit_av(*pending.pop(0))
            for _ in range(4):
                nc.tensor.matmul(wu_ps2[:, 0, :], wu_l[:], wu_r[:],
                                 start=True, stop=True)
            emit_av(*pending.pop(0))
            while chain_q:
                den_chain(*chain_q.pop(0)[1])
            for _ in range(12):
                nc.tensor.matmul(wu_ps2[:, 0, :], wu_l[:], wu_r[:],
                                 start=True, stop=True)
            nc.vector.tensor_copy(wu_sb[:], wu_ps2[ds(0, 1), 0, ds(0, 1)])
            for f in fill01 + fill23:
                f()

            # tail: Wo of the last i-group
            for itl in range(NI):
                it = 4 * (NI - 1) + itl
                for ncol in range(NO):
                    wo_unit(it, ncol)()


def _build_program():
    nc = bacc.Bacc("TRN2", target_bir_lowering=False, debug=False,
                   num_devices=NCORES)
    xT = nc.dram_tensor("xT", (D, S), MMDT, kind="ExternalInput").ap()
    wq = nc.dram_tensor("wq", (D, GW), MMDT, kind="ExternalInput").ap()
    wk = nc.dram_tensor("wk", (D, GW), MMDT, kind="ExternalInput").ap()
    wv = nc.dram_tensor("wv", (D, GW), MMDT, kind="ExternalInput").ap()
    wo = nc.dram_tensor("wo", (GW, D), MMDT, kind="ExternalInput").ap()
    bq = nc.dram_tensor("bq", (GW,), F32, kind="ExternalInput").ap()
    bk = nc.dram_tensor("bk", (GW,), F32, kind="ExternalInput").ap()
    bv = nc.dram_tensor("bv", (GW,), F32, kind="ExternalInput").ap()
    out = nc.dram_tensor("out", (S, D), MMDT, kind="ExternalOutput").ap()
    with tile.TileContext(nc) as tc:
        _mha_core(tc, out, xT, wq, wk, wv, wo, bq, bk, bv)
    nc.compile()
    return nc


_program = None


def _get_program():
    global _program
    if _program is None:
        _program = _build_program()
    return _program


def make_in_maps(x, Wq, bq, Wk, bk, Wv, bv, Wo, bo):
    in_maps = []
    f = np.float32
    bf = mybir.dt.np(MMDT)
    for c in range(NCORES):
        b, g = divmod(c, 4)
        sl = slice(g * GW, (g + 1) * GW)
        in_maps.append({
            "xT": np.ascontiguousarray(x[b].T).astype(bf),
            "wq": np.ascontiguousarray(Wq[:, sl]).astype(bf),
            "wk": np.ascontiguousarray(Wk[:, sl]).astype(bf),
            "wv": np.ascontiguousarray(Wv[:, sl]).astype(bf),
            "wo": np.ascontiguousarray(Wo[sl, :]).astype(bf),
            "bq": np.ascontiguousarray(bq[sl], dtype=f),
            "bk": np.ascontiguousarray(bk[sl], dtype=f),
            "bv": np.ascontiguousarray(bv[sl], dtype=f),
        })
    return in_maps


def run(inputs, trace=False, tmpdir=None, **kw):
    nc = _get_program()
    in_maps = make_in_maps(**inputs)
    res = run_bass_kernel_spmd(nc, in_maps, core_ids=list(range(NCORES)),
                               trace=trace, tmpdir=tmpdir, **kw)
    bo = inputs["bo"].astype(np.float32)
    parts = [np.asarray(res.results[c]["out"], dtype=np.float32)
             for c in range(NCORES)]
    y = np.stack(
        [parts[4 * b] + parts[4 * b + 1] + parts[4 * b + 2] + parts[4 * b + 3] + bo
         for b in range(B)], axis=0)
    return y.astype(np.float32), res


def kernel(**inputs):
    y, _ = run(inputs, trace=False)
    return y


# revision 25
# speedup vs baseline: 1.0496x; 1.0059x over previous
"""Multi-head attention (B=2, S=2048, D=1024, H=16, K=64) on 8 TRN2 cores.

Sharding: core c -> batch b=c//4, head-group g=c%4 (4 heads, 256-wide slice
of Wq/Wk/Wv columns and Wo rows).  Each core computes a partial (2048, 1024)
output in bf16; host sums groups of 4 cores in f32 and adds bo.

Per-core layout (all transposed so no on-chip transposes are needed):
  - host supplies xT = x[b].T  (D, S), bf16
  - Q^T, K^T computed as [gw_col, S] via lhsT=W chunk, rhs=xT chunk
  - scores^T[j, i] via lhsT=K^T chunk, rhs=Q^T into double-buffered 2-bank
    PSUM tiles; one ScalarE Exp covers 1024 elements
  - softmax denominator via a ones column appended to V (V_aug); probs are
    exp(scores/8) with no max subtraction (scores ~N(0,1), no overflow)
  - O^T = V_aug^T @ probs^T; division by the denominator happens on VectorE
    with head-pair batching (one reciprocal per pair, broadcast matmuls to
    the two column groups of one PSUM bank)

Scheduling: the attention inner loop alone cannot keep TensorE busy (the
Exp on ScalarE is the per-iteration rate limiter), and PE micro-idles make
the HAM clock-gate re-throttle the array to 1.2 GHz.  So the Wo matmuls of
the previous i-group and the Q^T projection of the next i-group are pumped
into the PE queue as filler work between score groups, keeping the PE
saturated and warm.  AV matmuls are emitted one score-group behind so the
PE never head-of-line blocks on the Exp.

All matmul operands are bf16 (PSUM accumulation stays fp32).
"""

import os
import sys
from contextlib import ExitStack

import numpy as np

if "/opt/trn_rl_repo" not in sys.path:
    sys.path.insert(0, "/opt/trn_rl_repo")

import concourse.bass as bass
import concourse.mybir as mybir
import concourse.tile as tile
from concourse import bacc
from concourse.bass import ds, ts
from concourse.bass_utils import run_bass_kernel_spmd

B, S, D = 2, 2048, 1024
H, KS = 16, 64
NCORES = 8
HPC = H // 4          # 4 heads per core
GW = HPC * KS         # 256-wide head-group slice
P = 128
ND = D // P           # 8 contraction chunks over d_model
NM = GW // P          # 2 col chunks of the group slice
NI = 4                # i-groups
IT = S // NI          # 512 rows per i-group
NJ = S // P           # 16 j-chunks
NJJ = 2               # j-chunks per Exp batch ([128,1024] ACT, 2 PSUM banks)
NG = NJ // NJJ        # score groups per head
NO = D // 512         # 2 out-col groups for Wo

F32 = mybir.dt.float32
BF16 = mybir.dt.bfloat16
MMDT = BF16
EXP = mybir.ActivationFunctionType.Exp


def _mha_core(tc, out, xT, wq, wk, wv, wo, bq, bk, bv):
    nc = tc.nc
    with ExitStack() as ctx:
        cp = ctx.enter_context(tc.tile_pool(name="const", bufs=1))
        probs_pool = ctx.enter_context(tc.tile_pool(name="probs", bufs=5))
        out_pool = ctx.enter_context(tc.tile_pool(name="outsb", bufs=3))
        den_pool = ctx.enter_context(tc.tile_pool(name="den", bufs=2))

        # ---- ACT table preload: tiny exp before anything else on ScalarE ----
        warm = cp.tile([1, 16], F32)
        nc.vector.memset(warm[:], 0.0)
        nc.scalar.activation(warm[:], warm[:], EXP)

        # HAM warmup operands: junk matmuls keep the PE busy while DMAs
        # land (and during the tail den-chain) so real matmuls run at
        # 2.4 GHz instead of the cold 1.2 GHz
        wu_l = cp.tile([P, P], MMDT)
        wu_r = cp.tile([P, IT], MMDT)
        nc.vector.memset(wu_l[:], 0.0)
        nc.vector.memset(wu_r[:], 0.0)

        # ---- inputs to SBUF; sync ring carries what's needed first ----
        wk_sb = cp.tile([P, ND, GW], MMDT)
        nc.sync.dma_start(wk_sb[:], wk.rearrange("(nd p) n -> p nd n", p=P))
        xT_sb = []
        for c4 in range(4):
            xc = cp.tile([P, ND, IT], MMDT, name=f"xc{c4}")
            nc.sync.dma_start(
                xc[:], xT[:, ts(c4, IT)].rearrange("(nd p) s -> p nd s", p=P))
            xT_sb.append(xc)
        wq_sb = cp.tile([P, ND, GW], MMDT)
        nc.sync.dma_start(wq_sb[:], wq.rearrange("(nd p) n -> p nd n", p=P))
        wv_sb = cp.tile([P, ND, GW], MMDT)
        wo_sb = cp.tile([P, NM, D], MMDT)
        nc.scalar.dma_start(wv_sb[:], wv.rearrange("(nd p) n -> p nd n", p=P))
        nc.scalar.dma_start(wo_sb[:], wo.rearrange("(nm p) n -> p nm n", p=P))
        bq_sb = cp.tile([P, NM], F32)
        bk_sb = cp.tile([P, NM], F32)
        nc.scalar.dma_start(bq_sb[:], bq.rearrange("(m p) -> p m", p=P))
        nc.scalar.dma_start(bk_sb[:], bk.rearrange("(m p) -> p m", p=P))
        bv_bc = cp.tile([P, GW], F32)
        nc.scalar.dma_start(bv_bc[:], bv.partition_broadcast(P))

        QT = cp.tile([P, NM, S], MMDT)
        KT = cp.tile([P, NM, S], MMDT)
        OT = cp.tile([P, NM, S], MMDT)
        # V_aug[:, h, jt, 0:64] = V rows, [:, h, jt, 64] = 1.0 (denominator col)
        V_aug = cp.tile([P, HPC, NJ, KS + 1], MMDT)
        nc.vector.memset(
            V_aug[:, :, :, ds(KS, 1)].rearrange("p h j o -> p (h j o)"), 1.0)

        # ones row for the denominator broadcast matmul (1/den -> 64 rows)
        ones64 = cp.tile([1, KS], MMDT)
        nc.vector.memset(ones64[:], 1.0)

        # ---- projections: K^T (ig-major, chasing the xT DMAs), V, Q^T(0) ----
        with tc.tile_pool(name="ps_acc", bufs=4, space="PSUM") as ps_acc:
            wu_ps = ps_acc.tile([P, IT], F32, tag="acc")
            for _ in range(12):
                nc.tensor.matmul(wu_ps[:], wu_l[:], wu_r[:],
                                 start=True, stop=True)
            wu_sb = cp.tile([1, 1], F32)
            nc.vector.tensor_copy(wu_sb[:], wu_ps[ds(0, 1), ds(0, 1)])
            for c4 in range(NI):
                for m in range(NM):
                    kt_ps = ps_acc.tile([P, IT], F32, tag="acc")
                    for dc in range(ND):
                        nc.tensor.matmul(
                            kt_ps[:],
                            wk_sb[:, dc, ts(m, P)],
                            xT_sb[c4][:, dc, :],
                            start=(dc == 0), stop=(dc == ND - 1),
                        )
                    nc.vector.tensor_scalar_add(
                        KT[:, m, ts(c4, IT)], kt_ps[:], bk_sb[:, ds(m, 1)])
                for jt in range(4 * c4, 4 * c4 + 4):
                    v_ps = ps_acc.tile([P, IT], F32, tag="acc")
                    for dc in range(ND):
                        nc.tensor.matmul(
                            v_ps[:, 0:GW],
                            xT_sb[jt // 4][:, dc, ts(jt % 4, P)],
                            wv_sb[:, dc, :],
                            start=(dc == 0), stop=(dc == ND - 1),
                        )
                    nc.vector.tensor_add(
                        V_aug[:, :, jt, 0:KS],
                        v_ps[:, 0:GW].rearrange("p (h k) -> p h k", h=HPC),
                        bv_bc[:].rearrange("p (h k) -> p h k", h=HPC),
                    )

            for m in range(NM):
                qt_ps = ps_acc.tile([P, IT], F32, tag="acc")
                for dc in range(ND):
                    nc.tensor.matmul(
                        qt_ps[:],
                        wq_sb[:, dc, ts(m, P)],
                        xT_sb[0][:, dc, :],
                        start=(dc == 0), stop=(dc == ND - 1),
                    )
                nc.vector.tensor_scalar_add(
                    QT[:, m, ts(0, IT)], qt_ps[:], bq_sb[:, ds(m, 1)])

        # ---- attention + interleaved Wo / Q^T-projection filler ----
        # PSUM budget: s2 = 2x2, o_ps = 2, misc(w/qt/bc) = 2 shared slots
        with tc.tile_pool(name="ps_s", bufs=2, space="PSUM") as ps_s, \
             tc.tile_pool(name="ps_o", bufs=2, space="PSUM") as ps_o, \
             tc.tile_pool(name="ps_m", bufs=2, space="PSUM") as ps_m:
            ps_c = ps_m

            stage, stage_left = {}, {}

            def wo_unit(it, ncol):
                def emit():
                    g = it // NI
                    if g not in stage and g != NI - 1:
                        stage[g] = out_pool.tile([P, NI, D], MMDT,
                                                 name="ostage", tag="ost",
                                                 bufs=2)
                        stage_left[g] = NI * NO
                    w_ps = ps_m.tile([P, 512], F32, tag="m")
                    for hc in range(NM):
                        nc.tensor.matmul(
                            w_ps[:],
                            OT[:, hc, ts(it, P)],
                            wo_sb[:, hc, ts(ncol, 512)],
                            start=(hc == 0), stop=(hc == NM - 1),
                        )
                    if g == NI - 1:
                        o_sb = out_pool.tile([P, 512], MMDT)
                        r = (it + ncol) % 3
                        if r == 1:
                            nc.scalar.copy(o_sb[:], w_ps[:])
                            nc.scalar.dma_start(
                                out[ts(it, P), ts(ncol, 512)], o_sb[:])
                        else:
                            nc.vector.tensor_copy(o_sb[:], w_ps[:])
                            eng = nc.sync if r == 0 else nc.gpsimd
                            eng.dma_start(
                                out[ts(it, P), ts(ncol, 512)], o_sb[:])
                        return
                    st = stage[g]
                    nc.vector.tensor_copy(st[:, it % NI, ts(ncol, 512)],
                                          w_ps[:])
                    stage_left[g] -= 1
                    if stage_left[g] == 0:
                        eng = nc.sync if g % 2 == 0 else nc.scalar
                        eng.dma_start(
                            out[ts(g, 4 * P), :].rearrange(
                                "(itl p) d -> p itl d", p=P),
                            st[:])
                        del stage[g]
                return emit

            def qt_units(g, m):
                """Q^T projection of i-group g, col chunk m, as 4 filler
                units of 2 accumulating matmuls each."""
                state = {}

                def unit(k):
                    def emit():
                        if k == 0:
                            state["ps"] = ps_m.tile([P, IT], F32,
                                                    name="qt_fill_ps", tag="m")
                        qt_ps = state["ps"]
                        for dc in (2 * k, 2 * k + 1):
                            nc.tensor.matmul(
                                qt_ps[:],
                                wq_sb[:, dc, ts(m, P)],
                                xT_sb[g][:, dc, :],
                                start=(dc == 0), stop=(dc == ND - 1),
                            )
                        if k == 3:
                            nc.vector.tensor_scalar_add(
                                QT[:, m, ts(g, IT)], qt_ps[:],
                                bq_sb[:, ds(m, 1)])
                    return emit
                return [unit(k) for k in range(4)]

            def den_chain(ig, m, oe_sb, oo_sb, den2):
                """Normalize heads 2m, 2m+1 of i-group ig from their SBUF
                evacuations (o rows in oe_sb/oo_sb, denominators in den2)."""
                recip2 = den_pool.tile([1, 2, IT], F32)
                nc.vector.reciprocal_approx_fast(
                    recip2[:].rearrange("p a b -> p (a b)"),
                    den2[:].rearrange("p a b -> p (a b)"))
                recip2b = den_pool.tile([1, 2, IT], MMDT)
                nc.vector.tensor_copy(
                    recip2b[:].rearrange("p a b -> p (a b)"),
                    recip2[:].rearrange("p a b -> p (a b)"))
                bc_ps = ps_c.tile([P, IT], F32, tag="m")
                nc.tensor.matmul(bc_ps[ds(0, KS), :], ones64[:],
                                 recip2b[:, 0, :], start=True, stop=True)
                nc.tensor.matmul(bc_ps[ds(KS, KS), :], ones64[:],
                                 recip2b[:, 1, :], start=True, stop=True,
                                 tile_position=(0, KS))
                nc.vector.tensor_mul(
                    OT[ds(0, KS), m, ts(ig, IT)], oe_sb[:],
                    bc_ps[ds(0, KS), :])
                nc.vector.tensor_mul(
                    OT[ds(KS, KS), m, ts(ig, IT)], oo_sb[:],
                    bc_ps[ds(KS, KS), :])

            # ---- flat software pipeline over (ig, pair, jc) ----
            # The two heads of a pair occupy row groups 0-63 / 64-127, so
            # their score matmuls run CONCURRENTLY in the PE array (row
            # tiling) and one Exp covers both heads' scores.
            o_tiles = {}
            fill01, fill23 = [], []

            def build_fillers(ig):
                f01, f23 = [], []
                if ig == 0:
                    f01 = qt_units(1, 0) + qt_units(1, 1)
                else:
                    f01 = [wo_unit(4 * (ig - 1) + itl, ncol)
                           for itl in range(NI) for ncol in range(NO)]
                    if ig < NI - 1:
                        f23 = qt_units(ig + 1, 0) + qt_units(ig + 1, 1)
                return f01, f23

            def emit_av(pig, pm, pjc, ppt):
                for par in range(2):
                    key = (pig, 2 * pm + par)
                    if key not in o_tiles:
                        o_tiles[key] = ps_o.tile([KS + 1, IT], F32,
                                                 name="o_ps", tag="o")
                    nc.tensor.matmul(
                        o_tiles[key][:], V_aug[:, 2 * pm + par, pjc, :],
                        ppt[:, par, :],
                        start=(pjc == 0), stop=(pjc == NJ - 1),
                    )
                if pjc == NJ - 1:
                    # evacuate both heads to SBUF (fast PSUM release);
                    # the pair normalization chain is deferred so its
                    # broadcast matmul never blocks the PE on the
                    # reciprocal latency
                    den2 = den_pool.tile([1, 2, IT], F32)
                    evs = []
                    for par in range(2):
                        o_full = o_tiles.pop((pig, 2 * pm + par))
                        o_sb = den_pool.tile([KS, IT], F32, name="o_evac",
                                             tag=f"oev{par}", bufs=2)
                        nc.vector.tensor_copy(o_sb[:], o_full[ds(0, KS), :])
                        nc.vector.tensor_copy(den2[:, par, :],
                                              o_full[ds(KS, 1), :])
                        evs.append(o_sb)
                    chain_q.append([2, (pig, pm, evs[0], evs[1], den2)])

            pending = []  # (ig, m, jc, pt), AV emitted at depth 2
            chain_q = []  # deferred pair normalization chains
            for ig in range(NI):
                for f in fill01 + fill23:
                    f()
                fill01, fill23 = build_fillers(ig)
                for pr in range(NM):
                    fillers = fill01 if pr == 0 else fill23
                    for jc in range(NJ):
                        sP = ps_s.tile([P, NJJ, IT], F32, tag="s")
                        nc.tensor.matmul(
                            sP[:, 0, :],
                            KT[ds(0, KS), pr, ts(jc, P)],
                            QT[ds(0, KS), pr, ts(ig, IT)],
                            start=True, stop=True,
                        )
                        nc.tensor.matmul(
                            sP[:, 1, :],
                            KT[ds(KS, KS), pr, ts(jc, P)],
                            QT[ds(KS, KS), pr, ts(ig, IT)],
                            start=True, stop=True,
                        )
                        lag = 2 if (ig == NI - 1 and pr == NM - 1
                                    and jc >= NJ - 4) else 3
                        if len(pending) >= lag:
                            emit_av(*pending.pop(0))
                        if chain_q:
                            chain_q[0][0] -= 1
                            if chain_q[0][0] <= 0:
                                den_chain(*chain_q.pop(0)[1])
                        # Wo fillers read OT, written by the deferred chain
                        # (emitted by jc==3 of the next pair) -> pump late;
                        # QT fillers have no such hazard -> front-load them
                        # so QT is ready well before the next i-group
                        if pr == 0:
                            if jc % 2 == 1 and jc >= 3 and fillers:
                                fillers.pop(0)()
                                if jc == NJ - 1 and fillers:
                                    fillers.pop(0)()
                        elif jc % 2 == 1 and jc < 8:
                            for _ in range(2):
                                if fillers:
                                    fillers.pop(0)()
                        pt = probs_pool.tile([P, NJJ, IT], MMDT)
                        nc.scalar.activation(
                            pt[:].rearrange("p a b -> p (a b)"),
                            sP[:].rearrange("p a b -> p (a b)"),
                            EXP, scale=0.125)
                        pending.append((ig, pr, jc, pt))
            # drain; junk matmuls keep the PE warm through the last chain
            wu_ps2 = ps_s.tile([P, NJJ, IT], F32, name="wu_ps2", tag="s")
            emit_av(*pending.pop(0))
            for _ in range(4):
                nc.tensor.matmul(wu_ps2[:, 0, :], wu_l[:], wu_r[:],
                                 start=True, stop=True)
            emit_av(*pending.pop(0))
            for _ in range(4):
                nc.tensor.matmul(wu_ps2[:, 0, :], wu_l[:], wu_r[:],
                                 start=True, stop=True)
            emit_av(*pending.pop(0))
            while chain_q:
                den_chain(*chain_q.pop(0)[1])
            for _ in range(6):
                nc.tensor.matmul(wu_ps2[:, 0, :], wu_l[:], wu_r[:],
                                 start=True, stop=True)
            nc.vector.tensor_copy(wu_sb[:], wu_ps2[ds(0, 1), 0, ds(0, 1)])
            for f in fill01 + fill23:
                f()

            # tail: Wo of the last i-group
            for itl in range(NI):
                it = 4 * (NI - 1) + itl
                for ncol in range(NO):
                    wo_unit(it, ncol)()


def _build_program():
    nc = bacc.Bacc("TRN2", target_bir_lowering=False, debug=False,
                   num_devices=NCORES)
    xT = nc.dram_tensor("xT", (D, S), MMDT, kind="ExternalInput").ap()
    wq = nc.dram_tensor("wq", (D, GW), MMDT, kind="ExternalInput").ap()
    wk = nc.dram_tensor("wk", (D, GW), MMDT, kind="ExternalInput").ap()
    wv = nc.dram_tensor("wv", (D, GW), MMDT, kind="ExternalInput").ap()
    wo = nc.dram_tensor("wo", (GW, D), MMDT, kind="ExternalInput").ap()
    bq = nc.dram_tensor("bq", (GW,), F32, kind="ExternalInput").ap()
    bk = nc.dram_tensor("bk", (GW,), F32, kind="ExternalInput").ap()
    bv = nc.dram_tensor("bv", (GW,), F32, kind="ExternalInput").ap()
    out = nc.dram_tensor("out", (S, D), MMDT, kind="ExternalOutput").ap()
    with tile.TileContext(nc) as tc:
        _mha_core(tc, out, xT, wq, wk, wv, wo, bq, bk, bv)
    nc.compile()
    return nc


_program = None


def _get_program():
    global _program
    if _program is None:
        _program = _build_program()
    return _program


def make_in_maps(x, Wq, bq, Wk, bk, Wv, bv, Wo, bo):
    in_maps = []
    f = np.float32
    bf = mybir.dt.np(MMDT)
    for c in range(NCORES):
        b, g = divmod(c, 4)
        sl = slice(g * GW, (g + 1) * GW)
        in_maps.append({
            "xT": np.ascontiguousarray(x[b].T).astype(bf),
            "wq": np.ascontiguousarray(Wq[:, sl]).astype(bf),
            "wk": np.ascontiguousarray(Wk[:, sl]).astype(bf),
            "wv": np.ascontiguousarray(Wv[:, sl]).astype(bf),
            "wo": np.ascontiguousarray(Wo[sl, :]).astype(bf),
            "bq": np.ascontiguousarray(bq[sl], dtype=f),
            "bk": np.ascontiguousarray(bk[sl], dtype=f),
            "bv": np.ascontiguousarray(bv[sl], dtype=f),
        })
    return in_maps


def run(inputs, trace=False, tmpdir=None, **kw):
    nc = _get_program()
    in_maps = make_in_maps(**inputs)
    res = run_bass_kernel_spmd(nc, in_maps, core_ids=list(range(NCORES)),
                               trace=trace, tmpdir=tmpdir, **kw)
    bo = inputs["bo"].astype(np.float32)
    parts = [np.asarray(res.results[c]["out"], dtype=np.float32)
             for c in range(NCORES)]
    y = np.stack(
        [parts[4 * b] + parts[4 * b + 1] + parts[4 * b + 2] + parts[4 * b + 3] + bo
         for b in range(B)], axis=0)
    return y.astype(np.float32), res


def kernel(**inputs):
    y, _ = run(inputs, trace=False)
    return y
